# revision 1
# baseline (speedup 1.0000x reference)
"""GATv2 2-layer + global-mean-pool classifier on 8 Trainium2 NeuronCores.

Strategy (1D node partitioning, dst-sharded edges):
  - 50000 nodes sharded contiguously across 8 cores (6250 each, padded to 6272).
  - Within each core, nodes are sorted by (padded) in-degree and grouped into
    49 supertiles of 128 nodes; each node's incident edges are padded to the
    supertile max degree D_t.  Layout: node-on-partition, degree slots along
    the free dimension.
  - Per supertile: one batched indirect DMA gathers xl~[src] rows (528B/272B)
    for all 128*D_t edge slots from a table in DRAM.
  - e = att.LeakyReLU(xl_j+xr_i) is computed via the identity
        e = 0.6*(attl_j+attr_i) + 0.4*sum_c sign(att_c)*|xl~_c + xr~_c|
    with table columns pre-scaled by |att_c| and permuted so positive /
    negative sign blocks are contiguous (abs folded into tensor_reduce).
  - Softmax division is deferred past the segment sum:
        sum_j alpha*xl_j = (sum_j exp(e_j)*xl_j) / sum_j exp(e_j)
    so the weighted sum is D_t PSUM-accumulated identity-lhsT matmuls.
  - The layer-2 source table xl2~ = h1 @ Wl2~ is computed shard-locally and
    AllGather'd across the 8 cores.
  - Global mean pool partials [64,64] per core via batch-onehot matmuls; the
    final tiny (64x64)@(64x10) linear runs on host.
"""

import os
import sys

import numpy as np

sys.path.insert(0, "/opt/trn_rl_repo")

# ---------------------------------------------------------------- constants
N = 50000
E = 600000
F_IN = 128
HID = 64
NC_CLS = 10
NG = 64
NEG = 0.2          # leaky relu slope -> Lrelu(z) = 0.6 z + 0.4 |z|
NCORES = 8
NSH_R = N // NCORES          # 6250 real nodes per core
NT = (NSH_R + 127) // 128    # 49 supertiles
NSH = NT * 128               # 6272 padded rank slots per core
TBL_N = NCORES * NSH         # 50176 table rows
MASK_NEG = -30.0             # pad-slot bias before exp (exp(-30)~1e-13, no inf/nan)
F1 = 132                     # L1 table row: 128 feats | attl(2) | pad(2)
F2 = 68                      # L2 table row: 64 feats | attl2(1) | pad(3)


def _sign_split(att_row, W, scale_floor=1e-8):
    """Column permutation + |att| scaling for one head.

    Returns (perm, k_pos, W_scaled_permuted, scales_permuted, att_perm)."""
    pos = np.where(att_row >= 0)[0]
    neg = np.where(att_row < 0)[0]
    perm = np.concatenate([pos, neg])
    scales = np.maximum(np.abs(att_row[perm]), scale_floor).astype(np.float32)
    Wsp = (W[:, perm] * scales[None, :]).astype(np.float32)
    return perm, len(pos), Wsp, scales


def prep(inputs):
    """All host-side restructuring. Returns (static, in_maps, host_ctx)."""
    x = np.asarray(inputs["x"], np.float32)
    ei = np.asarray(inputs["edge_index"], np.int64)
    batch = np.asarray(inputs["batch"], np.int64)
    Wl1 = np.asarray(inputs["Wl1"], np.float32)
    Wr1 = np.asarray(inputs["Wr1"], np.float32)
    att1 = np.asarray(inputs["att1"], np.float32)
    b1 = np.asarray(inputs["b1"], np.float32)
    Wl2 = np.asarray(inputs["Wl2"], np.float32)
    Wr2 = np.asarray(inputs["Wr2"], np.float32)
    att2 = np.asarray(inputs["att2"], np.float32)
    b2 = np.asarray(inputs["b2"], np.float32)

    loops = np.arange(N, dtype=np.int64)
    src = np.concatenate([ei[0], loops]).astype(np.int64)
    dst = np.concatenate([ei[1], loops]).astype(np.int64)

    # ---- per-head sign-split + scaling (layer 1) --------------------------
    P1 = np.zeros(2 * HID, np.int64)       # device col -> original col
    k1 = np.zeros(2, np.int64)
    Wl1s = np.zeros((F_IN, 2 * HID), np.float32)
    Wr1s = np.zeros((F_IN, 2 * HID), np.float32)
    inv1 = np.zeros(2 * HID, np.float32)   # descale per device col
    for h in range(2):
        blk = slice(h * HID, (h + 1) * HID)
        perm, kp, Wsp, scales = _sign_split(att1[h], Wl1[:, blk])
        _, _, Wsp_r, _ = _sign_split(att1[h], Wr1[:, blk])
        P1[blk] = h * HID + perm
        k1[h] = kp
        Wl1s[:, blk] = Wsp
        Wr1s[:, blk] = Wsp_r
        inv1[blk] = 1.0 / scales
    wattl1 = np.stack([Wl1[:, h * HID:(h + 1) * HID] @ att1[h] for h in range(2)], 1)
    wattr1 = np.stack([Wr1[:, h * HID:(h + 1) * HID] @ att1[h] for h in range(2)], 1)
    Wlp1 = np.concatenate([Wl1s, wattl1, np.zeros((F_IN, 2), np.float32)], 1)  # [128,132]
    Wrp1 = np.concatenate([Wr1s, wattr1, np.zeros((F_IN, 2), np.float32)], 1)

    # ---- layer 2 (heads=1); Wl2 rows permuted to device h1 order ----------
    Wl2d = Wl2[P1, :]
    Wr2d = Wr2[P1, :]
    P2, k2, Wl2s, scales2 = _sign_split(att2[0], Wl2d)
    _, _, Wr2s, _ = _sign_split(att2[0], Wr2d)
    inv2 = (1.0 / scales2).astype(np.float32)
    wattl2 = (Wl2d @ att2[0])[:, None]
    wattr2 = (Wr2d @ att2[0])[:, None]
    Wlp2 = np.concatenate([Wl2s, wattl2, np.zeros((2 * HID, 3), np.float32)], 1)  # [128,68]
    Wrp2 = np.concatenate([Wr2s, wattr2, np.zeros((2 * HID, 3), np.float32)], 1)

    # ---- shard + degree-sort + supertile structure ------------------------
    core_of = dst // NSH_R                       # 0..7
    order = np.argsort(core_of * np.int64(N) + dst, kind="stable")
    src_s, dst_s = src[order], dst[order]
    core_starts = np.searchsorted(core_of[order], np.arange(NCORES + 1))

    deg = np.bincount(dst, minlength=N)
    assert deg.max() <= 128, f"max degree {deg.max()} > 128"

    perm_nodes = np.zeros((NCORES, NSH), np.int64)   # rank -> global node id
    rank_of = np.zeros(N, np.int64)                  # global id -> rank in core
    Dt = np.zeros((NCORES, NT), np.int64)
    for c in range(NCORES):
        ids = np.arange(c * NSH_R, (c + 1) * NSH_R)
        p = np.argsort(-deg[ids], kind="stable")
        pn = ids[p]
        perm_nodes[c, :NSH_R] = pn
        perm_nodes[c, NSH_R:] = pn[0]                # pad ranks: dummy (unused)
        rank_of[pn] = np.arange(NSH_R)
        dg = deg[pn].reshape(-1)
        for t in range(NT):
            lo, hi = t * 128, min((t + 1) * 128, NSH_R)
            Dt[c, t] = max(int(dg[lo:hi].max()) if hi > lo else 1, 1)
    D = np.maximum(Dt.max(0), 1)                     # shared across cores
    SD = int(D.sum())
    off = np.concatenate([[0], np.cumsum(D)]).astype(np.int64)

    # table position of each global node id
    tbl_pos = (core_of_n := np.arange(N) // NSH_R) * NSH + rank_of  # [N]

    # ---- per-core gather idx + mask ---------------------------------------
    idx_h = np.zeros((NCORES, 128, SD), np.int32)
    mask_h = np.full((NCORES, 128, SD), MASK_NEG, np.float32)
    batch_h = np.full((NCORES, 128, NT), -1.0, np.float32)
    xT_h = np.zeros((NCORES, F_IN, NSH), np.float32)
    for c in range(NCORES):
        e0, e1 = core_starts[c], core_starts[c + 1]
        s_c, d_c = src_s[e0:e1], dst_s[e0:e1]
        r_c = rank_of[d_c]                     # rank of dst within core
        eo = np.argsort(r_c, kind="stable")
        s_c, r_c = s_c[eo], r_c[eo]
        # slot within node = running count
        starts = np.searchsorted(r_c, np.arange(NSH + 1))
        slot = np.arange(len(r_c)) - starts[r_c]
        t_of = r_c // 128
        p_of = r_c % 128
        idx_h[c, p_of, off[t_of] + slot] = tbl_pos[s_c].astype(np.int32)
        mask_h[c, p_of, off[t_of] + slot] = 0.0
        rb = np.full(NSH, -1.0, np.float32)
        rb[:NSH_R] = batch[perm_nodes[c, :NSH_R]].astype(np.float32)
        batch_h[c] = rb.reshape(NT, 128).T
        xT_h[c] = x[perm_nodes[c]].T
        xT_h[c, :, NSH_R:] = 0.0

    const_row = lambda v, w: np.tile(np.asarray(v, np.float32)[None, :], (128, 1))
    static = dict(D=[int(d) for d in D], SD=SD)
    common = {
        "wlp1": Wlp1, "wrp1": Wrp1, "wlp2": Wlp2, "wrp2": Wrp2,
        "attinv1": const_row(inv1, 128), "b1t": const_row(b1[P1], 128),
        "attinv2": const_row(inv2, 64), "b2t": const_row(b2[P2], 64),
        "iota64": np.tile(np.arange(64, dtype=np.float32)[None, :], (128, 1)),
        "ident": np.eye(128, dtype=np.float32),
    }
    in_maps = []
    for c in range(NCORES):
        m = dict(common)
        m["xT"] = xT_h[c]
        m["idx"] = idx_h[c]
        m["maskt"] = mask_h[c]
        m["batchv"] = batch_h[c]
        in_maps.append(m)

    host_ctx = dict(
        batch=batch, P2=P2, k1=[int(v) for v in k1], k2=int(k2),
        Wlin=np.asarray(inputs["Wlin"], np.float32),
        blin=np.asarray(inputs["blin"], np.float32),
    )
    static.update(k1=[int(v) for v in k1], k2=int(k2))
    return static, in_maps, host_ctx


def host_epilogue(partials, host_ctx):
    pooled = np.sum(np.stack(partials, 0), 0)                 # [64, 64] perm2 cols
    counts = np.bincount(host_ctx["batch"], minlength=NG).astype(np.float32)
    g = pooled / np.maximum(counts, 1.0)[:, None]
    Wlin_p = host_ctx["Wlin"][host_ctx["P2"], :]
    return (g @ Wlin_p + host_ctx["blin"]).astype(np.float32)


# ---------------------------------------------------------------- numpy mock
def numpy_device_mock(static, in_maps, host_ctx, want_inter=False):
    """Bit-faithful (fp32, same op order-ish) simulation of the device kernel.
    Used to validate all the host-side restructuring before touching HW."""
    D, SD = static["D"], static["SD"]
    off = np.concatenate([[0], np.cumsum(D)]).astype(np.int64)
    k1, k2 = static["k1"], static["k2"]
    partials = []
    # global tables built exactly like the device does (shard + allgather)
    tbl1 = np.zeros((TBL_N, F1), np.float32)
    xre1 = np.zeros((NCORES, 128, NT * F1), np.float32)
    for c, m in enumerate(in_maps):
        for t in range(NT):
            xsl = m["xT"][:, t * 128:(t + 1) * 128]          # [128c, 128n]
            tbl1[c * NSH + t * 128:c * NSH + (t + 1) * 128] = xsl.T @ m["wlp1"]
            xre1[c, :, t * F1:(t + 1) * F1] = xsl.T @ m["wrp1"]

    h1_all = np.zeros((NCORES, 128, NT * 128), np.float32)
    for c, m in enumerate(in_maps):
        for t in range(NT):
            d = D[t]
            idx = m["idx"][:, off[t]:off[t] + d]              # [128, d]
            msk = m["maskt"][:, off[t]:off[t] + d]
            A = tbl1[idx.reshape(-1)].reshape(128, d, F1)
            xr = xre1[c, :, t * F1:(t + 1) * F1]              # [128, 132]
            s = A + xr[:, None, :]
            e = np.zeros((128, 2, d), np.float32)
            for h in range(2):
                base = h * 64
                pos = np.abs(s[:, :, base:base + k1[h]]).sum(2)
                neg = np.abs(s[:, :, base + k1[h]:base + 64]).sum(2)
                attl = A[:, :, 128 + h]
                attr = xr[:, 128 + h]
                e[:, h] = 0.6 * (attl + attr[:, None]) + 0.4 * (pos - neg) + msk
            p = np.exp(e)
            denom = p.sum(2)                                   # [128, 2]
            outw = np.zeros((128, 128), np.float32)
            for dd in range(d):
                w = np.concatenate(
                    [A[:, dd, 0:64] * p[:, 0, dd:dd + 1],
                     A[:, dd, 64:128] * p[:, 1, dd:dd + 1]], 1)
                outw += w
            hh = np.concatenate([outw[:, 0:64] / denom[:, 0:1],
                                 outw[:, 64:128] / denom[:, 1:2]], 1)
            hh = hh * m["attinv1"] + m["b1t"]
            hh = np.maximum(hh, np.exp(np.minimum(hh, 0.0)) - 1.0)
            h1_all[c, :, t * 128:(t + 1) * 128] = hh

    tbl2 = np.zeros((TBL_N, F2), np.float32)
    xre2 = np.zeros((NCORES, 128, NT * F2), np.float32)
    for c, m in enumerate(in_maps):
        for t in range(NT):
            h1t = h1_all[c, :, t * 128:(t + 1) * 128]          # [128n, 128f]
            tbl2[c * NSH + t * 128:c * NSH + (t + 1) * 128] = h1t @ m["wlp2"]
            xre2[c, :, t * F2:(t + 1) * F2] = h1t @ m["wrp2"]

    h2_all = np.zeros((NCORES, 128, NT * 64), np.float32)
    for c, m in enumerate(in_maps):
        pooled = np.zeros((64, 64), np.float32)
        for t in range(NT):
            d = D[t]
            idx = m["idx"][:, off[t]:off[t] + d]
            msk = m["maskt"][:, off[t]:off[t] + d]
            A = tbl2[idx.reshape(-1)].reshape(128, d, F2)
            xr = xre2[c, :, t * F2:(t + 1) * F2]
            s = A + xr[:, None, :]
            pos = np.abs(s[:, :, 0:k2]).sum(2)
            neg = np.abs(s[:, :, k2:64]).sum(2)
            e = 0.6 * (A[:, :, 64] + xr[:, 64][:, None]) + 0.4 * (pos - neg) + msk
            p = np.exp(e)
            denom = p.sum(1)
            outw = (A[:, :, 0:64] * p[:, :, None]).sum(1)
            hh = outw / denom[:, None] * m["attinv2"] + m["b2t"]
            hh = np.maximum(hh, np.exp(np.minimum(hh, 0.0)) - 1.0)
            h2_all[c, :, t * 64:(t + 1) * 64] = hh
            onehot = (m["iota64"] == m["batchv"][:, t:t + 1]).astype(np.float32)
            pooled += onehot.T @ hh
        partials.append(pooled)
    out = host_epilogue(partials, host_ctx)
    if want_inter:
        return out, dict(tbl1=tbl1, xre1=xre1, h1=h1_all, tbl2=tbl2,
                         xre2=xre2, h2=h2_all, partials=partials)
    return out


# ---------------------------------------------------------------- device impl
def build_nc(static, debug=False):
    import concourse.bass as bass
    import concourse.bacc as bacc
    import concourse.mybir as mybir
    import concourse.tile as tile
    from contextlib import ExitStack

    fp32 = mybir.dt.float32
    i32 = mybir.dt.int32
    AF = mybir.ActivationFunctionType
    OP = mybir.AluOpType

    D, SD = static["D"], static["SD"]
    off = np.concatenate([[0], np.cumsum(D)]).astype(np.int64)
    k1, k2 = static["k1"], static["k2"]

    nc = bacc.Bacc(None, num_devices=NCORES)

    # ---- I/O ----
    xT = nc.dram_tensor("xT", [F_IN, NSH], fp32, kind="ExternalInput")
    wlp1 = nc.dram_tensor("wlp1", [F_IN, F1], fp32, kind="ExternalInput")
    wrp1 = nc.dram_tensor("wrp1", [F_IN, F1], fp32, kind="ExternalInput")
    wlp2 = nc.dram_tensor("wlp2", [2 * HID, F2], fp32, kind="ExternalInput")
    wrp2 = nc.dram_tensor("wrp2", [2 * HID, F2], fp32, kind="ExternalInput")
    idx = nc.dram_tensor("idx", [128, SD], i32, kind="ExternalInput")
    maskt = nc.dram_tensor("maskt", [128, SD], fp32, kind="ExternalInput")
    batchv = nc.dram_tensor("batchv", [128, NT], fp32, kind="ExternalInput")
    attinv1 = nc.dram_tensor("attinv1", [128, 128], fp32, kind="ExternalInput")
    b1t = nc.dram_tensor("b1t", [128, 128], fp32, kind="ExternalInput")
    attinv2 = nc.dram_tensor("attinv2", [128, 64], fp32, kind="ExternalInput")
    b2t = nc.dram_tensor("b2t", [128, 64], fp32, kind="ExternalInput")
    iota64 = nc.dram_tensor("iota64", [128, 64], fp32, kind="ExternalInput")
    ident = nc.dram_tensor("ident", [128, 128], fp32, kind="ExternalInput")
    pooled_out = nc.dram_tensor("pooled", [64, 64], fp32, kind="ExternalOutput")
    if debug:
        dbg_xre1 = nc.dram_tensor("dbg_xre1", [128, NT * F1], fp32,
                                  kind="ExternalOutput")
        dbg_h1 = nc.dram_tensor("dbg_h1", [128, NT * 128], fp32,
                                kind="ExternalOutput")
        dbg_h2 = nc.dram_tensor("dbg_h2", [128, NT * 64], fp32,
                                kind="ExternalOutput")
        dbg_tbl1 = nc.dram_tensor("dbg_tbl1", [256, F1], fp32,
                                  kind="ExternalOutput")
        dbg_tbl2 = nc.dram_tensor("dbg_tbl2", [256, F2], fp32,
                                  kind="ExternalOutput")
        D0 = static["D"][0]
        dbg_A = nc.dram_tensor("dbg_A", [128, D0 * F1], fp32, kind="ExternalOutput")
        dbg_s = nc.dram_tensor("dbg_s", [128, D0 * F1], fp32, kind="ExternalOutput")
        dbg_ew = nc.dram_tensor("dbg_ew", [128, 4 * D0], fp32, kind="ExternalOutput")
        dbg_e = nc.dram_tensor("dbg_e", [128, 2 * D0], fp32, kind="ExternalOutput")
        dbg_px = nc.dram_tensor("dbg_px", [128, 2 * D0], fp32, kind="ExternalOutput")
        dbg_den = nc.dram_tensor("dbg_den", [128, 2], fp32, kind="ExternalOutput")
        dbg_W = nc.dram_tensor("dbg_W", [128, D0 * 128], fp32, kind="ExternalOutput")
        dbg_po = nc.dram_tensor("dbg_po", [128, 128], fp32, kind="ExternalOutput")
        dbg = dict(A=dbg_A, s=dbg_s, ew=dbg_ew, e=dbg_e, px=dbg_px,
                   den=dbg_den, W=dbg_W, po=dbg_po)
    else:
        dbg = None

    # collective buffers (internal DRAM)
    tbl1_sh = nc.dram_tensor("tbl1_sh", [NSH, F1], fp32)
    tbl1 = nc.dram_tensor("tbl1", [TBL_N, F1], fp32, addr_space="Shared")
    tbl2_sh = nc.dram_tensor("tbl2_sh", [NSH, F2], fp32)
    tbl2 = nc.dram_tensor("tbl2", [TBL_N, F2], fp32, addr_space="Shared")

    with tile.TileContext(nc) as tc, ExitStack() as ctx:
        cp = ctx.enter_context(tc.tile_pool(name="const", bufs=1))
        # persistent sbuf buffers
        wlp1_s = cp.tile([F_IN, F1], fp32); nc.sync.dma_start(wlp1_s[:], wlp1[:, :])
        wrp1_s = cp.tile([F_IN, F1], fp32); nc.sync.dma_start(wrp1_s[:], wrp1[:, :])
        wlp2_s = cp.tile([2 * HID, F2], fp32); nc.sync.dma_start(wlp2_s[:], wlp2[:, :])
        wrp2_s = cp.tile([2 * HID, F2], fp32); nc.sync.dma_start(wrp2_s[:], wrp2[:, :])
        mask_s = cp.tile([128, SD], fp32); nc.sync.dma_start(mask_s[:], maskt[:, :])
        batch_s = cp.tile([128, NT], fp32); nc.sync.dma_start(batch_s[:], batchv[:, :])
        ai1_s = cp.tile([128, 128], fp32); nc.sync.dma_start(ai1_s[:], attinv1[:, :])
        b1_s = cp.tile([128, 128], fp32); nc.sync.dma_start(b1_s[:], b1t[:, :])
        ai2_s = cp.tile([128, 64], fp32); nc.sync.dma_start(ai2_s[:], attinv2[:, :])
        b2_s = cp.tile([128, 64], fp32); nc.sync.dma_start(b2_s[:], b2t[:, :])
        io64_s = cp.tile([128, 64], fp32); nc.sync.dma_start(io64_s[:], iota64[:, :])
        id_s = cp.tile([128, 128], fp32); nc.sync.dma_start(id_s[:], ident[:, :])

        big = ctx.enter_context(tc.tile_pool(name="big", bufs=1))
        xre1_s = big.tile([128, NT * F1], fp32)
        h1_s = big.tile([128, NT * 128], fp32)

        # ---------------- phase A: layer-1 tables ----------------
        with tc.tile_pool(name="phA", bufs=3) as pa, \
             tc.tile_pool(name="phA_ps", bufs=3, space="PSUM") as pap, \
             tc.tile_pool(name="xt", bufs=1) as pxt:
            xT_s = pxt.tile([F_IN, NSH], fp32)
            nc.sync.dma_start(xT_s[:], xT[:, :])
            for t in range(NT):
                lhs = xT_s[:, t * 128:(t + 1) * 128]
                ps = pap.tile([128, F1], fp32, tag="psA")
                nc.tensor.matmul(ps[:], lhs, wlp1_s[:], start=True, stop=True)
                stg = pa.tile([128, F1], fp32, tag="stgA")
                nc.scalar.copy(stg[:], ps[:])
                nc.sync.dma_start(tbl1_sh[t * 128:(t + 1) * 128, :], stg[:])
                ps2 = pap.tile([128, F1], fp32, tag="psA")
                nc.tensor.matmul(ps2[:], lhs, wrp1_s[:], start=True, stop=True)
                nc.scalar.copy(xre1_s[:, t * F1:(t + 1) * F1], ps2[:])

        nc.gpsimd.collective_compute(
            "AllGather", mybir.AluOpType.bypass,
            replica_groups=[list(range(NCORES))],
            ins=[tbl1_sh[:, :]], outs=[tbl1[:, :]],
        )

        # ---------------- phase B: layer-1 edges ----------------
        def edge_layer(tblT, xre_s, Fw, nheads, kpos, ai_s, bt_s, h_out, h_w,
                       dbgl=None):
            """Process all supertiles of one GATv2 layer.
            tblT: gather table dram AP;  xre_s: [128, NT*Fw] sbuf (dst transform)
            Fw: table row width;  kpos: list of pos-block sizes per head
            h_out: [128, NT*h_w] output sbuf tile."""
            maxD = max(D)
            with tc.tile_pool(name=f"edg{Fw}", bufs=2) as pe, \
                 tc.tile_pool(name=f"sm{Fw}", bufs=3) as psm, \
                 tc.tile_pool(name=f"ps{Fw}", bufs=2, space="PSUM") as pps:
                for t in range(NT):
                    d = D[t]
                    # idx/out for the indirect DMA must be exact contiguous
                    # tiles (sliced/strided APs crash the DMA engine)
                    idxt = pe.tile([128, d], i32, tag="idxt")
                    nc.sync.dma_start(idxt[:], idx[:, int(off[t]):int(off[t]) + d])
                    A = pe.tile([128, d * Fw], fp32, tag="A")
                    # HW indirect DMA honors ONE offset per partition per call
                    for kk in range(d):
                        nc.gpsimd.indirect_dma_start(
                            out=A[:, kk * Fw:(kk + 1) * Fw],
                            out_offset=None,
                            in_=tblT[:, :],
                            in_offset=bass.IndirectOffsetOnAxis(
                                ap=idxt[:, kk:kk + 1], axis=0),
                        )
                    A3 = A[:].rearrange("p (d f) -> p d f", f=Fw)
                    xr = xre_s[:, t * Fw:(t + 1) * Fw]
                    xrb = xr.rearrange("p (o f) -> p o f", o=1).to_broadcast(
                        [128, d, Fw])
                    s = pe.tile([128, maxD * Fw], fp32, tag="s")
                    s3 = s[:, :d * Fw].rearrange("p (d f) -> p d f", f=Fw)
                    nc.vector.tensor_tensor(s3, A3, xrb, op=OP.add)
                    # e-work tile: [pos_h, neg_h] x heads, then e [h, d]
                    ew = psm.tile([128, 4 * maxD], fp32, tag="ew")
                    for h in range(nheads):
                        base = h * 64
                        nc.vector.tensor_reduce(
                            ew[:, (2 * h) * d:(2 * h) * d + d],
                            s3[:, :, base:base + kpos[h]],
                            axis=mybir.AxisListType.X, op=OP.add,
                            apply_absolute_value=True)
                        nc.vector.tensor_reduce(
                            ew[:, (2 * h + 1) * d:(2 * h + 1) * d + d],
                            s3[:, :, base + kpos[h]:base + 64],
                            axis=mybir.AxisListType.X, op=OP.add,
                            apply_absolute_value=True)
                    # pn = pos - neg  -> [128, h, d]
                    pn = psm.tile([128, 2 * maxD], fp32, tag="pn")
                    ew4 = ew[:, :4 * d].rearrange("p (s d) -> p s d", d=d)
                    pnv = pn[:, :nheads * d].rearrange("p (s d) -> p s d", d=d)
                    nc.vector.tensor_tensor(
                        pnv, ew4[:, 0:2 * nheads:2, :], ew4[:, 1:2 * nheads:2, :],
                        op=OP.subtract)
                    # attr_mask = 0.6*attr + mask   [128, h, d]
                    am = psm.tile([128, 2 * maxD], fp32, tag="am")
                    amv = am[:, :nheads * d].rearrange("p (s d) -> p s d", d=d)
                    mvec = mask_s[:, int(off[t]):int(off[t]) + d]
                    mb = mvec.rearrange("p (o d) -> p o d", o=1).to_broadcast(
                        [128, nheads, d])
                    attr = xr[:, 128 if Fw == F1 else 64:][:, :nheads]
                    attrb = bass.AP(attr.tensor, attr.offset,
                                    [attr.ap[0], [1, nheads], [0, d]])
                    tmp = psm.tile([128, 2 * maxD], fp32, tag="amt")
                    tmpv = tmp[:, :nheads * d].rearrange("p (s d) -> p s d", d=d)
                    nc.vector.scalar_tensor_tensor(
                        tmpv, attrb, 0.6, mb, op0=OP.mult, op1=OP.add)
                    # e = (pn * 0.4) + tmp;  then += 0.6*attl
                    nc.vector.scalar_tensor_tensor(
                        amv, pnv, 0.4, tmpv, op0=OP.mult, op1=OP.add)
                    attl = A3[:, :, (128 if Fw == F1 else 64):(
                        128 if Fw == F1 else 64) + nheads]
                    attlv = bass.AP(A.tensor, A.offset + (128 if Fw == F1 else 64),
                                    [A.ap[0], [1, nheads], [Fw, d]])
                    ee = psm.tile([128, 2 * maxD], fp32, tag="ee")
                    eev = ee[:, :nheads * d].rearrange("p (s d) -> p s d", d=d)
                    nc.vector.scalar_tensor_tensor(
                        eev, attlv, 0.6, amv, op0=OP.mult, op1=OP.add)
                    # exp
                    pexp = psm.tile([128, 2 * maxD], fp32, tag="pexp")
                    pexpv = pexp[:, :nheads * d]
                    nc.scalar.activation(pexpv, ee[:, :nheads * d], AF.Exp)
                    pexp3 = pexpv.rearrange("p (s d) -> p s d", d=d)
                    # denom (fp32, exact) + recip
                    den = psm.tile([128, 2], fp32, tag="den")
                    nc.vector.tensor_reduce(den[:, :nheads], pexp3,
                                            axis=mybir.AxisListType.X, op=OP.add)
                    rd = psm.tile([128, 2], fp32, tag="rd")
                    nc.vector.reciprocal(rd[:, :nheads], den[:, :nheads])
                    # W = A * exp  (per head)
                    W = pe.tile([128, maxD * h_w], fp32, tag="W")
                    W3 = W[:, :d * h_w].rearrange("p (d f) -> p d f", f=h_w)
                    for h in range(nheads):
                        eb = bass.AP(pexp.tensor, pexp.offset + h * d,
                                     [pexp.ap[0], [1, d], [0, 64]])
                        nc.vector.tensor_tensor(
                            W3[:, :, h * 64:(h + 1) * 64],
                            A3[:, :, h * 64:(h + 1) * 64], eb, op=OP.mult)
                    # PSUM-accumulated identity matmuls over slots
                    po = pps.tile([128, h_w], fp32, tag="po")
                    for dd in range(d):
                        nc.tensor.matmul(po[:], id_s[:], W3[:, dd, :],
                                         start=(dd == 0), stop=(dd == d - 1))
                    if dbgl is not None and t == 0:
                        nc.sync.dma_start(dbgl["A"][:, :], A[:, :d * Fw])
                        nc.sync.dma_start(dbgl["s"][:, :], s[:, :d * Fw])
                        nc.sync.dma_start(dbgl["ew"][:, :], ew[:, :4 * d])
                        nc.sync.dma_start(dbgl["e"][:, :], ee[:, :nheads * d])
                        nc.sync.dma_start(dbgl["px"][:, :], pexp[:, :nheads * d])
                        nc.sync.dma_start(dbgl["den"][:, :], den[:, :])
                        nc.sync.dma_start(dbgl["W"][:, :], W[:, :d * h_w])
                        pstg = psm.tile([128, h_w], fp32, tag="pstg")
                        nc.scalar.copy(pstg[:], po[:])
                        nc.sync.dma_start(dbgl["po"][:, :], pstg[:])
                    # epilogue: divide by denom (ACT copy*scale), descale, bias, elu
                    hh = psm.tile([128, h_w], fp32, tag="hh")
                    for h in range(nheads):
                        nc.scalar.activation(
                            hh[:, h * 64:(h + 1) * 64], po[:, h * 64:(h + 1) * 64],
                            AF.Copy, bias=0.0, scale=rd[:, h:h + 1])
                    nc.vector.tensor_tensor(hh[:], hh[:], ai_s[:, :h_w], op=OP.mult)
                    nc.vector.tensor_tensor(hh[:], hh[:], bt_s[:, :h_w], op=OP.add)
                    # elu: max(x, exp(min(x,0)) - 1)
                    mn = psm.tile([128, h_w], fp32, tag="mn")
                    nc.vector.tensor_scalar(mn[:], hh[:], 0.0, None, op0=OP.min)
                    ex = psm.tile([128, h_w], fp32, tag="ex")
                    nc.scalar.activation(ex[:], mn[:], AF.Exp)
                    nc.vector.scalar_tensor_tensor(
                        h_out[:, t * h_w:(t + 1) * h_w], ex[:], -1.0, hh[:],
                        op0=OP.add, op1=OP.max)

        edge_layer(tbl1, xre1_s, F1, 2, k1, ai1_s, b1_s, h1_s, 128, dbgl=dbg)

        # ---------------- phase C: layer-2 tables ----------------
        xre2_s = big.tile([128, NT * F2], fp32)
        with tc.tile_pool(name="phC", bufs=3) as pc, \
             tc.tile_pool(name="phC_ps", bufs=3, space="PSUM") as pcp:
            for t in range(NT):
                psT = pcp.tile([128, 128], fp32, tag="psT")
                nc.tensor.transpose(psT[:], h1_s[:, t * 128:(t + 1) * 128], id_s[:])
                h1T = pc.tile([128, 128], fp32, tag="h1T")
                nc.scalar.copy(h1T[:], psT[:])
                ps = pcp.tile([128, F2], fp32, tag="psC")
                nc.tensor.matmul(ps[:], h1T[:], wlp2_s[:], start=True, stop=True)
                stg = pc.tile([128, F2], fp32, tag="stgC")
                nc.scalar.copy(stg[:], ps[:])
                nc.sync.dma_start(tbl2_sh[t * 128:(t + 1) * 128, :], stg[:])
                ps2 = pcp.tile([128, F2], fp32, tag="psC")
                nc.tensor.matmul(ps2[:], h1T[:], wrp2_s[:], start=True, stop=True)
                nc.scalar.copy(xre2_s[:, t * F2:(t + 1) * F2], ps2[:])

        nc.gpsimd.collective_compute(
            "AllGather", mybir.AluOpType.bypass,
            replica_groups=[list(range(NCORES))],
            ins=[tbl2_sh[:, :]], outs=[tbl2[:, :]],
        )

        # ---------------- phase D: layer-2 edges ----------------
        h2_s = big.tile([128, NT * 64], fp32)
        edge_layer(tbl2, xre2_s, F2, 1, [k2], ai2_s, b2_s, h2_s, 64)

        # ---------------- phase E: pooling ----------------
        with tc.tile_pool(name="phE", bufs=3) as pe_, \
             tc.tile_pool(name="phE_ps", bufs=1, space="PSUM") as pep:
            psP = pep.tile([64, 64], fp32)
            for t in range(NT):
                oh = pe_.tile([128, 64], fp32, tag="oh")
                nc.vector.tensor_scalar(oh[:], io64_s[:], batch_s[:, t:t + 1],
                                        None, op0=OP.is_equal)
                nc.tensor.matmul(psP[:], oh[:], h2_s[:, t * 64:(t + 1) * 64],
                                 start=(t == 0), stop=(t == NT - 1))
            stg = pe_.tile([64, 64], fp32, tag="stgE")
            nc.scalar.copy(stg[:], psP[:])
            nc.sync.dma_start(pooled_out[:, :], stg[:])

        if debug:
            with tc.tile_pool(name="dbg", bufs=2) as pd:
                nc.sync.dma_start(dbg_xre1[:, :], xre1_s[:])
                nc.sync.dma_start(dbg_h1[:, :], h1_s[:])
                nc.sync.dma_start(dbg_h2[:, :], h2_s[:])
                for j, (tbl, dbg, fw) in enumerate(
                        [(tbl1, dbg_tbl1, F1), (tbl2, dbg_tbl2, F2)]):
                    for half in range(2):
                        b = pd.tile([128, max(F1, F2)], fp32, tag="dbgb")
                        nc.sync.dma_start(
                            b[:, :fw], tbl[half * NSH:half * NSH + 128, :])
                        nc.sync.dma_start(
                            dbg[half * 128:(half + 1) * 128, :], b[:, :fw])

    nc.finalize()
    return nc


_CACHE = {}


def kernel(**inputs) -> np.ndarray:
    static, in_maps, host_ctx = prep(inputs)
    key = (tuple(static["D"]), tuple(static["k1"]), static["k2"])
    if key not in _CACHE:
        _CACHE[key] = build_nc(static)
    nc = _CACHE[key]
    from concourse.bass_utils import run_bass_kernel_spmd
    res = run_bass_kernel_spmd(nc, in_maps, core_ids=list(range(NCORES)))
    partials = [r["pooled"] for r in res.results]
    return host_epilogue(partials, host_ctx)



# revision 2
# speedup vs baseline: 2.2640x; 2.2640x over previous
"""GATv2 2-layer + global-mean-pool classifier on 8 Trainium2 NeuronCores.

Strategy (1D node partitioning, dst-sharded edges):
  - 50000 nodes sharded contiguously across 8 cores (6250 each, padded to 6272).
  - Within each core, nodes are sorted by in-degree and grouped into 49
    supertiles of 128 nodes; each node's incident edges are padded to the
    supertile max degree D_t.  Layout: node-on-partition, degree slots along
    the free dimension.
  - Per supertile: one batched indirect DMA per degree slot gathers xl~[src]
    rows (528B/272B) for all 128 nodes from a table in DRAM.
  - e = att.LeakyReLU(xl_j+xr_i) via the identity
        e = (0.6-scaled attl_j+attr_i cols) + sum_c 0.4|att_c|*|xl_c + xr_c|
    with columns sign-permuted so positive / negative blocks are contiguous
    (abs folded into tensor_reduce; the 0.4|att| scale applied on device in
    fp32 so tables stay unscaled).
  - Pad slots point at a poison table row whose attl cols are overwritten to
    -1e4 on device => exp underflows to exactly 0 (no mask tensor shipped);
    all-pad rows are saved from 0/0 by a denominator clamp.
  - Softmax division is deferred past the segment sum; the weighted sum is
    D_t PSUM-accumulated identity-lhsT matmuls.
  - The layer tables are computed shard-locally and AllGather'd.
  - Transfers over the axon tunnel are the wall-clock bottleneck: x ships as
    fp16 in transposed layout, edge indices as uint16, small consts packed
    into one row replicated on device via a rank-1 matmul.  All device_puts
    are issued asynchronously as soon as each host array is ready, and the
    jax/shard_map executable is built once and cached across calls.
"""

import sys

import numpy as np

sys.path.insert(0, "/opt/trn_rl_repo")

# ---------------------------------------------------------------- constants
N = 50000
E = 600000
F_IN = 128
HID = 64
NC_CLS = 10
NG = 64
NCORES = 8
NSH_R = N // NCORES          # 6250 real nodes per core
NT = (NSH_R + 127) // 128    # 49 supertiles
NSH = NT * 128               # 6272 padded rank slots per core
TBL_N = NCORES * NSH         # 50176 table rows
POISON = NSH_R               # local rank of the poison row (first pad rank)
F1 = 132                     # L1 table row: 128 feats | attl(2) | pad(2)
F2 = 68                      # L2 table row: 64 feats | attl2(1) | pad(3)
CP = F1 + 128 + F2 + 64      # cpack row: attsc1 | b1 | attsc2 | b2


# ---------------------------------------------------------------- host prep
def _prep_weights(inputs):
    att1 = np.asarray(inputs["att1"], np.float32)
    att2 = np.asarray(inputs["att2"], np.float32)
    Wl1 = np.asarray(inputs["Wl1"], np.float32)
    Wr1 = np.asarray(inputs["Wr1"], np.float32)
    Wl2 = np.asarray(inputs["Wl2"], np.float32)
    Wr2 = np.asarray(inputs["Wr2"], np.float32)
    b1 = np.asarray(inputs["b1"], np.float32)
    b2 = np.asarray(inputs["b2"], np.float32)

    P1 = np.zeros(2 * HID, np.int64)
    k1 = [0, 0]
    Wl1p = np.zeros((F_IN, 2 * HID), np.float32)
    Wr1p = np.zeros((F_IN, 2 * HID), np.float32)
    attsc1 = np.zeros(F1, np.float32)
    for h in (0, 1):
        a = att1[h]
        perm = np.concatenate([np.where(a >= 0)[0], np.where(a < 0)[0]])
        k1[h] = int((a >= 0).sum())
        blk = slice(h * HID, (h + 1) * HID)
        P1[blk] = h * HID + perm
        Wl1p[:, blk] = Wl1[:, blk][:, perm]
        Wr1p[:, blk] = Wr1[:, blk][:, perm]
        attsc1[h * HID:(h + 1) * HID] = 0.4 * np.abs(a[perm])
    wattl1 = 0.6 * np.stack([Wl1[:, h * HID:(h + 1) * HID] @ att1[h]
                             for h in (0, 1)], 1)
    wattr1 = 0.6 * np.stack([Wr1[:, h * HID:(h + 1) * HID] @ att1[h]
                             for h in (0, 1)], 1)
    z2 = np.zeros((F_IN, 2), np.float32)
    Wlp1 = np.concatenate([Wl1p, wattl1, z2], 1).astype(np.float16)
    Wrp1 = np.concatenate([Wr1p, wattr1, z2], 1).astype(np.float16)

    Wl2d = Wl2[P1, :]
    Wr2d = Wr2[P1, :]
    a2 = att2[0]
    P2 = np.concatenate([np.where(a2 >= 0)[0], np.where(a2 < 0)[0]])
    k2 = int((a2 >= 0).sum())
    attsc2 = np.zeros(F2, np.float32)
    attsc2[:HID] = 0.4 * np.abs(a2[P2])
    wattl2 = 0.6 * (Wl2d @ a2)[:, None]
    wattr2 = 0.6 * (Wr2d @ a2)[:, None]
    z3 = np.zeros((2 * HID, 3), np.float32)
    Wlp2 = np.concatenate([Wl2d[:, P2], wattl2, z3], 1).astype(np.float32)
    Wrp2 = np.concatenate([Wr2d[:, P2], wattr2, z3], 1).astype(np.float32)

    cpack = np.concatenate([attsc1, b1[P1], attsc2, b2[P2]]).astype(np.float32)
    return dict(Wlp1=Wlp1, Wrp1=Wrp1, Wlp2=Wlp2, Wrp2=Wrp2, cpack=cpack,
                P2=P2, k1=k1, k2=k2)


def _prep_graph(ei):
    """Degree-sort node partition + supertile degree profile."""
    src = np.concatenate([ei[0].astype(np.int32),
                          np.arange(N, dtype=np.int32)])
    dst = np.concatenate([ei[1].astype(np.int32),
                          np.arange(N, dtype=np.int32)])
    deg = np.bincount(dst, minlength=N).astype(np.int32)
    assert deg.max() <= 128, f"max degree {deg.max()} > 128"
    deg2 = deg.reshape(NCORES, NSH_R)
    order = np.argsort(-deg2, axis=1, kind="stable")
    degs = np.take_along_axis(deg2, order, axis=1)
    degsp = np.zeros((NCORES, NSH), np.int32)
    degsp[:, :NSH_R] = degs
    D = np.maximum(degsp[:, ::128].max(axis=0), 1)
    off = np.concatenate([[0], np.cumsum(D)]).astype(np.int64)
    perm_nodes = order + (np.arange(NCORES, dtype=np.int32) * NSH_R)[:, None]
    rank_of = np.empty(N, np.int32)
    rank_of[perm_nodes.ravel()] = np.tile(
        np.arange(NSH_R, dtype=np.int32), NCORES)
    return src, dst, D, off, perm_nodes, rank_of


def _build_xT(x, perm_nodes):
    x16 = np.asarray(x, np.float32).astype(np.float16)
    xT_cat = np.zeros((NCORES * F_IN, NSH), np.float16)
    for c in range(NCORES):
        xT_cat[c * F_IN:(c + 1) * F_IN, :NSH_R] = x16[perm_nodes[c]].T
    return xT_cat


def _build_edges(src, dst, D, off, rank_of, SD):
    gkey = (dst // NSH_R) * NSH + rank_of[dst]
    eorder = np.argsort(gkey, kind="stable")
    gs = gkey[eorder]
    vals = ((src // NSH_R) * NSH + rank_of[src])[eorder]
    starts = np.searchsorted(gs, np.arange(TBL_N + 1, dtype=np.int32))
    slot = np.arange(len(gs), dtype=np.int64) - starts[gs]
    c_e = gs // NSH
    r_e = gs % NSH
    idx_cat = np.full((NCORES * 128, SD), POISON, np.uint16)
    idx_cat[c_e * 128 + (r_e & 127), off[r_e >> 7] + slot] = \
        vals.astype(np.uint16)
    return idx_cat


def _build_batch(batch_np, perm_nodes):
    bpad = np.full((NCORES, NSH), -1.0, np.float32)
    bpad[:, :NSH_R] = batch_np[perm_nodes].astype(np.float32)
    return np.ascontiguousarray(
        bpad.reshape(NCORES, NT, 128).transpose(0, 2, 1)
    ).reshape(NCORES * 128, NT)


def prep(inputs):
    """Full host-side restructuring (single-shot path used by the mock)."""
    w = _prep_weights(inputs)
    ei = np.asarray(inputs["edge_index"])
    src, dst, D, off, perm_nodes, rank_of = _prep_graph(ei)
    SD = int(D.sum())
    static = dict(D=[int(d) for d in D], SD=SD, k1=w["k1"], k2=w["k2"])
    arrs = {
        "xT": _build_xT(inputs["x"], perm_nodes),
        "wlp1": np.tile(w["Wlp1"][None], (NCORES, 1, 1)).reshape(-1, F1),
        "wrp1": np.tile(w["Wrp1"][None], (NCORES, 1, 1)).reshape(-1, F1),
        "wlp2": np.tile(w["Wlp2"][None], (NCORES, 1, 1)).reshape(-1, F2),
        "wrp2": np.tile(w["Wrp2"][None], (NCORES, 1, 1)).reshape(-1, F2),
        "idxu": _build_edges(src, dst, D, off, rank_of, SD),
        "cpack": np.tile(w["cpack"][None], (NCORES, 1)),
        "batchv": _build_batch(np.asarray(inputs["batch"]).astype(np.int32),
                               perm_nodes),
    }
    host_ctx = dict(
        batch=np.asarray(inputs["batch"]).astype(np.int32), P2=w["P2"],
        Wlin=np.asarray(inputs["Wlin"], np.float32),
        blin=np.asarray(inputs["blin"], np.float32),
    )
    return static, arrs, host_ctx


def host_epilogue(pooled_global, host_ctx):
    pooled = pooled_global.reshape(NCORES, NG, HID).sum(0)
    counts = np.bincount(host_ctx["batch"], minlength=NG).astype(np.float32)
    g = pooled / np.maximum(counts, 1.0)[:, None]
    Wlin_p = host_ctx["Wlin"][host_ctx["P2"], :]
    return (g @ Wlin_p + host_ctx["blin"]).astype(np.float32)


# ---------------------------------------------------------------- numpy mock
def numpy_device_mock(static, arrs, host_ctx):
    """Bit-faithful-ish (fp32 with fp16-rounded inputs) simulation of the
    device kernel.  Used to validate host-side restructuring off-hardware."""
    D, SD = static["D"], static["SD"]
    off = np.concatenate([[0], np.cumsum(D)]).astype(np.int64)
    k1, k2 = static["k1"], static["k2"]
    xT = arrs["xT"].reshape(NCORES, F_IN, NSH).astype(np.float32)
    wlp1 = arrs["wlp1"][:F_IN].astype(np.float32)
    wrp1 = arrs["wrp1"][:F_IN].astype(np.float32)
    wlp2 = arrs["wlp2"][:2 * HID]
    wrp2 = arrs["wrp2"][:2 * HID]
    idx = arrs["idxu"].reshape(NCORES, 128, SD).astype(np.int64)
    cpk = arrs["cpack"][0]
    attsc1 = cpk[0:F1]
    b1r = cpk[F1:F1 + 128]
    attsc2 = cpk[F1 + 128:F1 + 128 + F2]
    b2r = cpk[F1 + 128 + F2:CP]
    batchv = arrs["batchv"].reshape(NCORES, 128, NT)

    def edge_layer(tbl, xre, Fw, nheads, kpos, attsc, brow, h_w):
        h_all = np.zeros((NCORES, 128, NT * h_w), np.float32)
        for c in range(NCORES):
            for t in range(NT):
                d = D[t]
                A = tbl[idx[c, :, off[t]:off[t] + d].reshape(-1)].reshape(
                    128, d, Fw)
                xr = xre[c, :, t * Fw:(t + 1) * Fw]
                s = (A + xr[:, None, :]) * attsc[None, None, :]
                e = np.zeros((128, nheads, d), np.float32)
                for h in range(nheads):
                    base = h * HID
                    pos = np.abs(s[:, :, base:base + kpos[h]]).sum(2)
                    neg = np.abs(s[:, :, base + kpos[h]:base + HID]).sum(2)
                    attl = A[:, :, h_w + h] if Fw == F1 else A[:, :, HID + h]
                    attr = xr[:, (128 if Fw == F1 else HID) + h]
                    e[:, h] = (attl + attr[:, None]) + (pos - neg)
                p = np.exp(e)
                den = np.maximum(p.sum(2), 1e-30)
                outw = np.zeros((128, h_w), np.float32)
                for h in range(nheads):
                    outw[:, h * HID:(h + 1) * HID] = (
                        A[:, :, h * HID:(h + 1) * HID]
                        * p[:, h, :, None]).sum(1) / den[:, h:h + 1]
                hh = outw + brow[None, :h_w]
                hh = np.maximum(hh, np.exp(np.minimum(hh, 0.0)) - 1.0)
                h_all[c, :, t * h_w:(t + 1) * h_w] = hh
        return h_all

    tbl1 = np.zeros((TBL_N, F1), np.float32)
    xre1 = np.zeros((NCORES, 128, NT * F1), np.float32)
    for c in range(NCORES):
        for t in range(NT):
            xsl = xT[c][:, t * 128:(t + 1) * 128]
            tbl1[c * NSH + t * 128:c * NSH + (t + 1) * 128] = xsl.T @ wlp1
            xre1[c, :, t * F1:(t + 1) * F1] = xsl.T @ wrp1
    tbl1[np.arange(NCORES) * NSH + POISON, 128:130] = -1e4
    h1 = edge_layer(tbl1, xre1, F1, 2, k1, attsc1, cpk[F1:F1 + 128], 128)

    tbl2 = np.zeros((TBL_N, F2), np.float32)
    xre2 = np.zeros((NCORES, 128, NT * F2), np.float32)
    for c in range(NCORES):
        for t in range(NT):
            h1t = h1[c, :, t * 128:(t + 1) * 128]
            tbl2[c * NSH + t * 128:c * NSH + (t + 1) * 128] = h1t @ wlp2
            xre2[c, :, t * F2:(t + 1) * F2] = h1t @ wrp2
    tbl2[np.arange(NCORES) * NSH + POISON, HID:HID + 1] = -1e4
    h2 = edge_layer(tbl2, xre2, F2, 1, [k2], attsc2, b2r, HID)

    pooled = np.zeros((NCORES, NG, HID), np.float32)
    for c in range(NCORES):
        for t in range(NT):
            onehot = (np.arange(NG, dtype=np.float32)[None, :]
                      == batchv[c, :, t:t + 1]).astype(np.float32)
            pooled[c] += onehot.T @ h2[c, :, t * HID:(t + 1) * HID]
    return host_epilogue(pooled.reshape(-1, HID), host_ctx)


# ---------------------------------------------------------------- device impl
def build_nc(static):
    import concourse.bass as bass
    import concourse.bacc as bacc
    import concourse.mybir as mybir
    import concourse.tile as tile
    from contextlib import ExitStack

    fp32 = mybir.dt.float32
    fp16 = mybir.dt.float16
    i32 = mybir.dt.int32
    u16 = mybir.dt.uint16
    AF = mybir.ActivationFunctionType
    OP = mybir.AluOpType

    D, SD = static["D"], static["SD"]
    off = np.concatenate([[0], np.cumsum(D)]).astype(np.int64)
    k1, k2 = static["k1"], static["k2"]

    nc = bacc.Bacc(None, num_devices=NCORES)

    # ---- I/O ----
    xT = nc.dram_tensor("xT", [F_IN, NSH], fp16, kind="ExternalInput")
    wlp1 = nc.dram_tensor("wlp1", [F_IN, F1], fp16, kind="ExternalInput")
    wrp1 = nc.dram_tensor("wrp1", [F_IN, F1], fp16, kind="ExternalInput")
    wlp2 = nc.dram_tensor("wlp2", [2 * HID, F2], fp32, kind="ExternalInput")
    wrp2 = nc.dram_tensor("wrp2", [2 * HID, F2], fp32, kind="ExternalInput")
    idxu = nc.dram_tensor("idxu", [128, SD], u16, kind="ExternalInput")
    cpack = nc.dram_tensor("cpack", [1, CP], fp32, kind="ExternalInput")
    batchv = nc.dram_tensor("batchv", [128, NT], fp32, kind="ExternalInput")
    pooled_out = nc.dram_tensor("pooled", [NG, HID], fp32,
                                kind="ExternalOutput")

    # collective buffers (internal DRAM)
    tbl1_sh = nc.dram_tensor("tbl1_sh", [NSH, F1], fp32)
    tbl1 = nc.dram_tensor("tbl1", [TBL_N, F1], fp32, addr_space="Shared")
    tbl2_sh = nc.dram_tensor("tbl2_sh", [NSH, F2], fp32)
    tbl2 = nc.dram_tensor("tbl2", [TBL_N, F2], fp32, addr_space="Shared")

    with tile.TileContext(nc) as tc, ExitStack() as ctx:
        cp = ctx.enter_context(tc.tile_pool(name="const", bufs=1))
        wlp1_s = cp.tile([F_IN, F1], fp16); nc.sync.dma_start(wlp1_s[:], wlp1[:, :])
        wrp1_s = cp.tile([F_IN, F1], fp16); nc.sync.dma_start(wrp1_s[:], wrp1[:, :])
        wlp2_s = cp.tile([2 * HID, F2], fp32); nc.sync.dma_start(wlp2_s[:], wlp2[:, :])
        wrp2_s = cp.tile([2 * HID, F2], fp32); nc.sync.dma_start(wrp2_s[:], wrp2[:, :])
        batch_s = cp.tile([128, NT], fp32); nc.sync.dma_start(batch_s[:], batchv[:, :])
        cpk_s = cp.tile([1, CP], fp32); nc.sync.dma_start(cpk_s[:], cpack[:, :])
        idxu_s = cp.tile([128, SD], u16); nc.sync.dma_start(idxu_s[:], idxu[:, :])
        idx32_s = cp.tile([128, SD], i32)
        nc.vector.tensor_scalar(idx32_s[:], idxu_s[:], 0, None, op0=OP.add)

        ones_s = cp.tile([1, 128], fp32); nc.vector.memset(ones_s[:], 1.0)
        pois_s = cp.tile([1, 2], fp32); nc.vector.memset(pois_s[:], -1e4)
        iotaF_i = cp.tile([128, 128], i32)
        nc.gpsimd.iota(iotaF_i[:], [[1, 128]], channel_multiplier=0)
        iotaP_i = cp.tile([128, 1], i32)
        nc.gpsimd.iota(iotaP_i[:], [[1, 1]], channel_multiplier=1)
        iotaF_f = cp.tile([128, 128], fp32)
        nc.vector.tensor_scalar(iotaF_f[:], iotaF_i[:], 0, None, op0=OP.add)
        iotaP_f = cp.tile([128, 1], fp32)
        nc.vector.tensor_scalar(iotaP_f[:], iotaP_i[:], 0, None, op0=OP.add)
        id_s = cp.tile([128, 128], fp32)
        nc.vector.tensor_scalar(id_s[:], iotaF_f[:], iotaP_f[:, 0:1], None,
                                op0=OP.is_equal)
        io64_s = iotaF_f[:, 0:NG]

        consts_s = cp.tile([128, CP], fp32)
        with tc.tile_pool(name="init_ps", bufs=1, space="PSUM") as ip:
            psC = ip.tile([128, CP], fp32)
            nc.tensor.matmul(psC[:], ones_s[:], cpk_s[:], start=True, stop=True)
            nc.scalar.copy(consts_s[:], psC[:])
        attsc1_s = consts_s[:, 0:F1]
        b1_s = consts_s[:, F1:F1 + 128]
        attsc2_s = consts_s[:, F1 + 128:F1 + 128 + F2]
        b2_s = consts_s[:, F1 + 128 + F2:CP]

        big = ctx.enter_context(tc.tile_pool(name="big", bufs=1))
        xre1_s = big.tile([128, NT * F1], fp32)
        h1_s = big.tile([128, NT * 128], fp32)

        # ---------------- phase A: layer-1 tables ----------------
        with tc.tile_pool(name="phA", bufs=3) as pa, \
             tc.tile_pool(name="phA_ps", bufs=3, space="PSUM") as pap, \
             tc.tile_pool(name="xt", bufs=1) as pxt:
            xT_s = pxt.tile([F_IN, NSH], fp16)
            nc.sync.dma_start(xT_s[:], xT[:, :])
            for t in range(NT):
                lhs = xT_s[:, t * 128:(t + 1) * 128]
                ps = pap.tile([128, F1], fp32, tag="psA")
                nc.tensor.matmul(ps[:], lhs, wlp1_s[:], start=True, stop=True)
                stg = pa.tile([128, F1], fp32, tag="stgA")
                nc.scalar.copy(stg[:], ps[:])
                nc.sync.dma_start(tbl1_sh[t * 128:(t + 1) * 128, :], stg[:])
                ps2 = pap.tile([128, F1], fp32, tag="psA")
                nc.tensor.matmul(ps2[:], lhs, wrp1_s[:], start=True, stop=True)
                nc.scalar.copy(xre1_s[:, t * F1:(t + 1) * F1], ps2[:])
        nc.sync.dma_start(tbl1_sh[POISON:POISON + 1, 128:130], pois_s[0:1, 0:2])

        nc.gpsimd.collective_compute(
            "AllGather", mybir.AluOpType.bypass,
            replica_groups=[list(range(NCORES))],
            ins=[tbl1_sh[:, :]], outs=[tbl1[:, :]],
        )

        # ---------------- edge phase ----------------
        def edge_layer(tblT, xre_s, Fw, nheads, kpos, attsc_s, bt_s, h_out,
                       h_w):
            maxD = max(D)
            with tc.tile_pool(name=f"edg{Fw}", bufs=2) as pe, \
                 tc.tile_pool(name=f"sm{Fw}", bufs=3) as psm, \
                 tc.tile_pool(name=f"ps{Fw}", bufs=2, space="PSUM") as pps:
                for t in range(NT):
                    d = D[t]
                    # idx/out for the indirect DMA must be exact contiguous
                    # tiles (sliced/strided APs crash the DMA engine)
                    idxt = pe.tile([128, d], i32, tag="idxt")
                    nc.vector.tensor_scalar(
                        idxt[:], idx32_s[:, int(off[t]):int(off[t]) + d],
                        0, None, op0=OP.add)
                    A = pe.tile([128, d * Fw], fp32, tag="A")
                    # HW indirect DMA honors ONE offset per partition per call
                    for kk in range(d):
                        nc.gpsimd.indirect_dma_start(
                            out=A[:, kk * Fw:(kk + 1) * Fw],
                            out_offset=None,
                            in_=tblT[:, :],
                            in_offset=bass.IndirectOffsetOnAxis(
                                ap=idxt[:, kk:kk + 1], axis=0),
                        )
                    A3 = A[:].rearrange("p (d f) -> p d f", f=Fw)
                    xr = xre_s[:, t * Fw:(t + 1) * Fw]
                    xrb = xr.rearrange("p (o f) -> p o f", o=1).to_broadcast(
                        [128, d, Fw])
                    s = pe.tile([128, maxD * Fw], fp32, tag="s")
                    s3 = s[:, :d * Fw].rearrange("p (d f) -> p d f", f=Fw)
                    nc.vector.tensor_tensor(s3, A3, xrb, op=OP.add)
                    ascb = attsc_s.rearrange("p (o f) -> p o f",
                                             o=1).to_broadcast([128, d, Fw])
                    nc.vector.tensor_tensor(s3, s3, ascb, op=OP.mult)
                    # e-work tile: [pos_h, neg_h] x heads, then e [h, d]
                    ew = psm.tile([128, 4 * maxD], fp32, tag="ew")
                    for h in range(nheads):
                        base = h * HID
                        nc.vector.tensor_reduce(
                            ew[:, (2 * h) * d:(2 * h) * d + d],
                            s3[:, :, base:base + kpos[h]],
                            axis=mybir.AxisListType.X, op=OP.add,
                            apply_absolute_value=True)
                        nc.vector.tensor_reduce(
                            ew[:, (2 * h + 1) * d:(2 * h + 1) * d + d],
                            s3[:, :, base + kpos[h]:base + HID],
                            axis=mybir.AxisListType.X, op=OP.add,
                            apply_absolute_value=True)
                    # pn = pos - neg  -> [128, h, d]
                    pn = psm.tile([128, 2 * maxD], fp32, tag="pn")
                    ew4 = ew[:, :4 * d].rearrange("p (s d) -> p s d", d=d)
                    pnv = pn[:, :nheads * d].rearrange("p (s d) -> p s d", d=d)
                    nc.vector.tensor_tensor(
                        pnv, ew4[:, 0:2 * nheads:2, :],
                        ew4[:, 1:2 * nheads:2, :], op=OP.subtract)
                    # e = (attl + attr) + pn   (0.6 folded into watt cols)
                    ac = 128 if Fw == F1 else HID
                    attr = xr[:, ac:][:, :nheads]
                    attrb = bass.AP(attr.tensor, attr.offset,
                                    [attr.ap[0], [1, nheads], [0, d]])
                    attlv = bass.AP(A.tensor, A.offset + ac,
                                    [A.ap[0], [1, nheads], [Fw, d]])
                    tmp = psm.tile([128, 2 * maxD], fp32, tag="tmp")
                    tmpv = tmp[:, :nheads * d].rearrange("p (s d) -> p s d", d=d)
                    nc.vector.tensor_tensor(tmpv, attlv, attrb, op=OP.add)
                    ee = psm.tile([128, 2 * maxD], fp32, tag="ee")
                    eev = ee[:, :nheads * d].rearrange("p (s d) -> p s d", d=d)
                    nc.vector.tensor_tensor(eev, tmpv, pnv, op=OP.add)
                    # exp
                    pexp = psm.tile([128, 2 * maxD], fp32, tag="pexp")
                    pexpv = pexp[:, :nheads * d]
                    nc.scalar.activation(pexpv, ee[:, :nheads * d], AF.Exp)
                    pexp3 = pexpv.rearrange("p (s d) -> p s d", d=d)
                    # denom + clamp (all-pad rows sum to exactly 0) + recip
                    den = psm.tile([128, 2], fp32, tag="den")
                    nc.vector.tensor_reduce(den[:, :nheads], pexp3,
                                            axis=mybir.AxisListType.X,
                                            op=OP.add)
                    rd = psm.tile([128, 2], fp32, tag="rd")
                    nc.vector.tensor_scalar(rd[:, :nheads], den[:, :nheads],
                                            1e-30, None, op0=OP.max)
                    nc.vector.reciprocal(rd[:, :nheads], rd[:, :nheads])
                    # W = A * exp  (per head)
                    W = pe.tile([128, maxD * h_w], fp32, tag="W")
                    W3 = W[:, :d * h_w].rearrange("p (d f) -> p d f", f=h_w)
                    for h in range(nheads):
                        eb = bass.AP(pexp.tensor, pexp.offset + h * d,
                                     [pexp.ap[0], [1, d], [0, HID]])
                        nc.vector.tensor_tensor(
                            W3[:, :, h * HID:(h + 1) * HID],
                            A3[:, :, h * HID:(h + 1) * HID], eb, op=OP.mult)
                    # PSUM-accumulated identity matmuls over slots
                    po = pps.tile([128, h_w], fp32, tag="po")
                    for dd in range(d):
                        nc.tensor.matmul(po[:], id_s[:], W3[:, dd, :],
                                         start=(dd == 0), stop=(dd == d - 1))
                    # epilogue: divide by denom (ACT copy*scale), bias, elu
                    hh = psm.tile([128, h_w], fp32, tag="hh")
                    for h in range(nheads):
                        nc.scalar.activation(
                            hh[:, h * HID:(h + 1) * HID],
                            po[:, h * HID:(h + 1) * HID],
                            AF.Copy, bias=0.0, scale=rd[:, h:h + 1])
                    nc.vector.tensor_tensor(hh[:], hh[:], bt_s[:, :h_w],
                                            op=OP.add)
                    # elu: max(x, exp(min(x,0)) - 1)
                    mn = psm.tile([128, h_w], fp32, tag="mn")
                    nc.vector.tensor_scalar(mn[:], hh[:], 0.0, None, op0=OP.min)
                    ex = psm.tile([128, h_w], fp32, tag="ex")
                    nc.scalar.activation(ex[:], mn[:], AF.Exp)
                    nc.vector.scalar_tensor_tensor(
                        h_out[:, t * h_w:(t + 1) * h_w], ex[:], -1.0, hh[:],
                        op0=OP.add, op1=OP.max)

        edge_layer(tbl1, xre1_s, F1, 2, k1, attsc1_s, b1_s, h1_s, 128)

        # ---------------- phase C: layer-2 tables ----------------
        xre2_s = big.tile([128, NT * F2], fp32)
        with tc.tile_pool(name="phC", bufs=3) as pc, \
             tc.tile_pool(name="phC_ps", bufs=3, space="PSUM") as pcp:
            for t in range(NT):
                psT = pcp.tile([128, 128], fp32, tag="psT")
                nc.tensor.transpose(psT[:], h1_s[:, t * 128:(t + 1) * 128],
                                    id_s[:])
                h1T = pc.tile([128, 128], fp32, tag="h1T")
                nc.scalar.copy(h1T[:], psT[:])
                ps = pcp.tile([128, F2], fp32, tag="psC")
                nc.tensor.matmul(ps[:], h1T[:], wlp2_s[:], start=True, stop=True)
                stg = pc.tile([128, F2], fp32, tag="stgC")
                nc.scalar.copy(stg[:], ps[:])
                nc.sync.dma_start(tbl2_sh[t * 128:(t + 1) * 128, :], stg[:])
                ps2 = pcp.tile([128, F2], fp32, tag="psC")
                nc.tensor.matmul(ps2[:], h1T[:], wrp2_s[:], start=True, stop=True)
                nc.scalar.copy(xre2_s[:, t * F2:(t + 1) * F2], ps2[:])
        nc.sync.dma_start(tbl2_sh[POISON:POISON + 1, HID:HID + 1],
                          pois_s[0:1, 0:1])

        nc.gpsimd.collective_compute(
            "AllGather", mybir.AluOpType.bypass,
            replica_groups=[list(range(NCORES))],
            ins=[tbl2_sh[:, :]], outs=[tbl2[:, :]],
        )

        # ---------------- phase D: layer-2 edges ----------------
        h2_s = big.tile([128, NT * HID], fp32)
        edge_layer(tbl2, xre2_s, F2, 1, [k2], attsc2_s, b2_s, h2_s, HID)

        # ---------------- phase E: pooling ----------------
        with tc.tile_pool(name="phE", bufs=3) as pe_, \
             tc.tile_pool(name="phE_ps", bufs=1, space="PSUM") as pep:
            psP = pep.tile([NG, HID], fp32)
            for t in range(NT):
                oh = pe_.tile([128, NG], fp32, tag="oh")
                nc.vector.tensor_scalar(oh[:], io64_s, batch_s[:, t:t + 1],
                                        None, op0=OP.is_equal)
                nc.tensor.matmul(psP[:], oh[:], h2_s[:, t * HID:(t + 1) * HID],
                                 start=(t == 0), stop=(t == NT - 1))
            stg = pe_.tile([NG, HID], fp32, tag="stgE")
            nc.scalar.copy(stg[:], psP[:])
            nc.sync.dma_start(pooled_out[:, :], stg[:])

    nc.finalize()
    return nc


# ---------------------------------------------------------------- runner
class _Runner:
    """Builds the Bass module + shard_map'd jit executable ONCE; later calls
    reuse it (no retracing).  device_put is async -> callers overlap H2D with
    the rest of host prep."""

    def __init__(self, static):
        import jax
        import concourse.mybir as mybir
        from jax.sharding import Mesh, PartitionSpec, NamedSharding
        from jax.experimental.shard_map import shard_map
        from concourse.bass2jax import (
            _bass_exec_p, partition_id_tensor, install_neuronx_cc_hook)

        install_neuronx_cc_hook()
        self.jax = jax
        nc = build_nc(static)
        self.nc = nc
        pname = nc.partition_id_tensor.name if nc.partition_id_tensor else None
        in_names, out_names, out_avals, zero_shapes = [], [], [], []
        for alloc in nc.m.functions[0].allocations:
            if not isinstance(alloc, mybir.MemoryLocationSet):
                continue
            name = alloc.memorylocations[0].name
            if alloc.kind == "ExternalInput":
                if name != pname:
                    in_names.append(name)
            elif alloc.kind == "ExternalOutput":
                shape = tuple(alloc.tensor_shape)
                dtype = mybir.dt.np(alloc.dtype)
                out_names.append(name)
                out_avals.append(jax.core.ShapedArray(shape, dtype))
                zero_shapes.append((shape, dtype))
        self.dbg_name = None
        if nc.dbg_addr is not None:
            assert not nc.dbg_callbacks
            self.dbg_name = nc.dbg_addr.name
            in_names.append(self.dbg_name)
        n_params = len(in_names)
        all_names = in_names + out_names + ([pname] if pname else [])
        self.in_names = in_names
        self.out_names = out_names
        self.zero_shapes = zero_shapes
        donate = tuple(range(n_params, n_params + len(out_names)))

        def _body(*args):
            operands = list(args)
            if pname is not None:
                operands.append(partition_id_tensor())
            return tuple(_bass_exec_p.bind(
                *operands, out_avals=tuple(out_avals),
                in_names=tuple(all_names), out_names=tuple(out_names),
                lowering_input_output_aliases=(),
                sim_require_finite=True, sim_require_nnan=True, nc=nc))

        devices = jax.devices()[:NCORES]
        mesh = Mesh(np.asarray(devices), ("core",))
        self.sharding = NamedSharding(mesh, PartitionSpec("core"))
        nio = n_params + len(out_names)
        self.fn = jax.jit(
            shard_map(_body, mesh=mesh,
                      in_specs=(PartitionSpec("core"),) * nio,
                      out_specs=(PartitionSpec("core"),) * len(out_names),
                      check_rep=False),
            donate_argnums=donate, keep_unused=True)

    def put(self, arr):
        return self.jax.device_put(arr, self.sharding)

    def run(self, handles):
        if self.dbg_name is not None and self.dbg_name not in handles:
            handles[self.dbg_name] = self.put(
                np.zeros((NCORES, 2), np.uint32))
        zeros = [self.put(np.zeros((NCORES * s[0],) + tuple(s[1:]), dt))
                 for s, dt in self.zero_shapes]
        outs = self.fn(*[handles[n] for n in self.in_names], *zeros)
        return np.asarray(outs[self.out_names.index("pooled")])


_CACHE = {}


def _get_runner(static):
    key = (tuple(static["D"]), tuple(static["k1"]), static["k2"])
    if key not in _CACHE:
        _CACHE[key] = _Runner(static)
    return _CACHE[key]


def kernel(**inputs) -> np.ndarray:
    w = _prep_weights(inputs)
    ei = np.asarray(inputs["edge_index"])
    src, dst, D, off, perm_nodes, rank_of = _prep_graph(ei)
    SD = int(D.sum())
    static = dict(D=[int(d) for d in D], SD=SD, k1=w["k1"], k2=w["k2"])
    runner = _get_runner(static)

    # big tensor first: device_put is async, the transfer streams while the
    # remaining host prep runs
    handles = {"xT": runner.put(_build_xT(inputs["x"], perm_nodes))}
    handles["idxu"] = runner.put(_build_edges(src, dst, D, off, rank_of, SD))
    handles["wlp1"] = runner.put(
        np.tile(w["Wlp1"][None], (NCORES, 1, 1)).reshape(-1, F1))
    handles["wrp1"] = runner.put(
        np.tile(w["Wrp1"][None], (NCORES, 1, 1)).reshape(-1, F1))
    handles["wlp2"] = runner.put(
        np.tile(w["Wlp2"][None], (NCORES, 1, 1)).reshape(-1, F2))
    handles["wrp2"] = runner.put(
        np.tile(w["Wrp2"][None], (NCORES, 1, 1)).reshape(-1, F2))
    handles["cpack"] = runner.put(np.tile(w["cpack"][None], (NCORES, 1)))
    batch_np = np.asarray(inputs["batch"]).astype(np.int32)
    handles["batchv"] = runner.put(_build_batch(batch_np, perm_nodes))

    pooled_global = runner.run(handles)
    host_ctx = dict(batch=batch_np, P2=w["P2"],
                    Wlin=np.asarray(inputs["Wlin"], np.float32),
                    blin=np.asarray(inputs["blin"], np.float32))
    return host_epilogue(pooled_global, host_ctx)


# revision 16
# speedup vs baseline: 4.8389x; 2.1373x over previous
"""GATv2 2-layer + global-mean-pool classifier on 8 Trainium2 NeuronCores.

Strategy (1D node partitioning, dst-sharded edges):
  - 50000 nodes sharded contiguously across 8 cores (6250 each, padded to 6272).
  - Within each core, nodes are sorted by in-degree and grouped into 49
    supertiles of 128 nodes; each node's incident edges are padded to the
    supertile max degree D_t.  Layout: node-on-partition, degree slots along
    the free dimension.
  - Per supertile: one batched indirect DMA per degree slot gathers xl~[src]
    rows (528B/272B) for all 128 nodes from a table in DRAM.
  - e = att.LeakyReLU(xl_j+xr_i) via the identity
        e = (0.6-scaled attl_j+attr_i cols) + sum_c 0.4|att_c|*|xl_c + xr_c|
    with columns sign-permuted so positive / negative blocks are contiguous
    (abs folded into tensor_reduce; the 0.4|att| scale applied on device in
    fp32 so tables stay unscaled).
  - Pad slots point at a poison table row whose attl cols are overwritten to
    -1e4 on device => exp underflows to exactly 0 (no mask tensor shipped);
    all-pad rows are saved from 0/0 by a denominator clamp.
  - Softmax division is deferred past the segment sum; the weighted sum is
    D_t PSUM-accumulated identity-lhsT matmuls.
  - The layer tables are computed shard-locally and AllGather'd.
  - Transfers over the axon tunnel are the wall-clock bottleneck: x ships as
    fp16 in transposed layout, edge indices as uint16, small consts packed
    into one row replicated on device via a rank-1 matmul.  All device_puts
    are issued asynchronously as soon as each host array is ready, and the
    jax/shard_map executable is built once and cached across calls.
"""

import sys

import ml_dtypes
import numpy as np

sys.path.insert(0, "/opt/trn_rl_repo")

F8 = ml_dtypes.float8_e4m3      # what mybir.dt.float8e4 maps to

# ---------------------------------------------------------------- constants
N = 50000
E = 600000
F_IN = 128
HID = 64
NC_CLS = 10
NG = 64
NCORES = 8
NSH_R = N // NCORES          # 6250 real nodes per core
NT = (NSH_R + 127) // 128    # 49 supertiles
NSH = NT * 128               # 6272 padded rank slots per core
TBL_N = NCORES * NSH         # 50176 table rows
POISON = NSH_R               # local rank of the poison row (first pad rank)
F1 = 132                     # L1 table row: 128 feats | attl(2) | pad(2)
F2 = 68                      # L2 table row: 64 feats | attl2(1) | pad(3)
CP = F1 + 128 + F2 + 64      # cpack row: attsc1 | b1 | attsc2 | b2


# ---------------------------------------------------------------- host prep
def _prep_weights(inputs):
    att1 = np.asarray(inputs["att1"], np.float32)
    att2 = np.asarray(inputs["att2"], np.float32)
    Wl1 = np.asarray(inputs["Wl1"], np.float32)
    Wr1 = np.asarray(inputs["Wr1"], np.float32)
    Wl2 = np.asarray(inputs["Wl2"], np.float32)
    Wr2 = np.asarray(inputs["Wr2"], np.float32)
    b1 = np.asarray(inputs["b1"], np.float32)
    b2 = np.asarray(inputs["b2"], np.float32)

    P1 = np.zeros(2 * HID, np.int64)
    k1 = [0, 0]
    Wl1p = np.zeros((F_IN, 2 * HID), np.float32)
    Wr1p = np.zeros((F_IN, 2 * HID), np.float32)
    attsc1 = np.zeros(F1, np.float32)
    for h in (0, 1):
        a = att1[h]
        perm = np.concatenate([np.where(a >= 0)[0], np.where(a < 0)[0]])
        k1[h] = int((a >= 0).sum())
        blk = slice(h * HID, (h + 1) * HID)
        P1[blk] = h * HID + perm
        Wl1p[:, blk] = Wl1[:, blk][:, perm]
        Wr1p[:, blk] = Wr1[:, blk][:, perm]
        attsc1[h * HID:(h + 1) * HID] = 0.4 * np.abs(a[perm])
    wattl1 = 0.6 * np.stack([Wl1[:, h * HID:(h + 1) * HID] @ att1[h]
                             for h in (0, 1)], 1)
    wattr1 = 0.6 * np.stack([Wr1[:, h * HID:(h + 1) * HID] @ att1[h]
                             for h in (0, 1)], 1)
    z2 = np.zeros((F_IN, 2), np.float32)
    Wlp1 = np.concatenate([Wl1p, wattl1, z2], 1).astype(np.float16)
    Wrp1 = np.concatenate([Wr1p, wattr1, z2], 1).astype(np.float16)

    Wl2d = Wl2[P1, :]
    Wr2d = Wr2[P1, :]
    a2 = att2[0]
    P2 = np.concatenate([np.where(a2 >= 0)[0], np.where(a2 < 0)[0]])
    k2 = int((a2 >= 0).sum())
    attsc2 = np.zeros(F2, np.float32)
    attsc2[:HID] = 0.4 * np.abs(a2[P2])
    wattl2 = 0.6 * (Wl2d @ a2)[:, None]
    wattr2 = 0.6 * (Wr2d @ a2)[:, None]
    z3 = np.zeros((2 * HID, 3), np.float32)
    Wlp2 = np.concatenate([Wl2d[:, P2], wattl2, z3], 1).astype(np.float32)
    Wrp2 = np.concatenate([Wr2d[:, P2], wattr2, z3], 1).astype(np.float32)

    cpack = np.concatenate([attsc1, b1[P1], attsc2, b2[P2]]).astype(np.float32)
    return dict(Wlp1=Wlp1, Wrp1=Wrp1, Wlp2=Wlp2, Wrp2=Wrp2, cpack=cpack,
                P2=P2, k1=k1, k2=k2)


def _prep_graph(ei):
    """Degree-sort node partition + supertile degree profile."""
    src = np.concatenate([ei[0].astype(np.int32),
                          np.arange(N, dtype=np.int32)])
    dst = np.concatenate([ei[1].astype(np.int32),
                          np.arange(N, dtype=np.int32)])
    deg = np.bincount(dst, minlength=N).astype(np.int32)
    assert deg.max() <= 128, f"max degree {deg.max()} > 128"
    deg2 = deg.reshape(NCORES, NSH_R)
    order = np.argsort(-deg2, axis=1, kind="stable")
    degs = np.take_along_axis(deg2, order, axis=1)
    degsp = np.zeros((NCORES, NSH), np.int32)
    degsp[:, :NSH_R] = degs
    D = np.maximum(degsp[:, ::128].max(axis=0), 1)
    off = np.concatenate([[0], np.cumsum(D)]).astype(np.int64)
    perm_nodes = order + (np.arange(NCORES, dtype=np.int32) * NSH_R)[:, None]
    rank_of = np.empty(N, np.int32)
    rank_of[perm_nodes.ravel()] = np.tile(
        np.arange(NSH_R, dtype=np.int32), NCORES)
    return src, dst, D, off, perm_nodes, rank_of


def _build_x8(x, perm_nodes):
    """Node features, degree-permuted, fp8-e4m3 wire format, [nodes, feat]
    layout (the device PE-transposes each 128x128 tile)."""
    x = np.asarray(x, np.float32)
    x8_cat = np.zeros((NCORES * NSH, F_IN), F8)
    for c in range(NCORES):
        x8_cat[c * NSH:c * NSH + NSH_R] = x[perm_nodes[c]].astype(F8)
    return x8_cat


def _build_edges(src, dst, D, off, rank_of, SD):
    gkey = ((dst // NSH_R) * NSH + rank_of[dst]).astype(np.uint16)
    eorder = np.argsort(gkey, kind="stable")   # 2-pass radix on u16
    gs = gkey[eorder].astype(np.int32)
    vals = ((src // NSH_R) * NSH + rank_of[src]).astype(np.uint16)[eorder]
    starts = np.searchsorted(gs, np.arange(TBL_N + 1, dtype=np.int32)
                             ).astype(np.int64)
    slot = np.arange(len(gs), dtype=np.int64) - starts[gs]
    c_e = gs // NSH
    r_e = gs % NSH
    idx_cat = np.full((NCORES * 128, SD), POISON, np.uint16)
    idx_cat[c_e * 128 + (r_e & 127), off[r_e >> 7] + slot] = vals
    return idx_cat


def _build_batch(batch_np, perm_nodes):
    bpad = np.full((NCORES, NSH), -1.0, np.float32)
    bpad[:, :NSH_R] = batch_np[perm_nodes].astype(np.float32)
    return np.ascontiguousarray(
        bpad.reshape(NCORES, NT, 128).transpose(0, 2, 1)
    ).reshape(NCORES * 128, NT)


def prep(inputs):
    """Full host-side restructuring (single-shot path used by the mock)."""
    w = _prep_weights(inputs)
    ei = np.asarray(inputs["edge_index"])
    src, dst, D, off, perm_nodes, rank_of = _prep_graph(ei)
    SD = int(D.sum())
    static = dict(D=[int(d) for d in D], SD=SD, k1=w["k1"], k2=w["k2"])
    arrs = {
        "x8": _build_x8(inputs["x"], perm_nodes),
        "wg16": np.tile(np.concatenate([w["Wlp1"], w["Wrp1"]], 1)[None],
                        (NCORES, 1, 1)).reshape(-1, 2 * F1),
        "wg32": np.concatenate([
            np.tile(np.concatenate([w["Wlp2"], w["Wrp2"]], 1)[None],
                    (NCORES, 1, 1)).reshape(-1, 2 * F2),
            _build_batch(np.asarray(inputs["batch"]).astype(np.int32),
                         perm_nodes)], 1),
        "idxu": _build_edges(src, dst, D, off, rank_of, SD),
        "cpack": np.tile(w["cpack"][None], (NCORES, 1)),
    }
    host_ctx = dict(
        batch=np.asarray(inputs["batch"]).astype(np.int32), P2=w["P2"],
        Wlin=np.asarray(inputs["Wlin"], np.float32),
        blin=np.asarray(inputs["blin"], np.float32),
    )
    return static, arrs, host_ctx


def host_epilogue(pooled, host_ctx):
    """pooled: [NG, HID] already summed across cores (device AllReduce)."""
    counts = np.bincount(host_ctx["batch"], minlength=NG).astype(np.float32)
    g = pooled / np.maximum(counts, 1.0)[:, None]
    Wlin_p = host_ctx["Wlin"][host_ctx["P2"], :]
    return (g @ Wlin_p + host_ctx["blin"]).astype(np.float32)


# ---------------------------------------------------------------- numpy mock
def numpy_device_mock(static, arrs, host_ctx):
    """Bit-faithful-ish (fp32 with fp16-rounded inputs) simulation of the
    device kernel.  Used to validate host-side restructuring off-hardware."""
    D, SD = static["D"], static["SD"]
    off = np.concatenate([[0], np.cumsum(D)]).astype(np.int64)
    k1, k2 = static["k1"], static["k2"]
    x16 = arrs["x8"].astype(np.float16)       # device widens fp8 -> fp16
    xT = np.ascontiguousarray(
        x16.reshape(NCORES, NSH, F_IN).transpose(0, 2, 1)).astype(np.float32)
    wlp1 = arrs["wg16"][:F_IN, 0:F1].astype(np.float32)
    wrp1 = arrs["wg16"][:F_IN, F1:2 * F1].astype(np.float32)
    wlp2 = arrs["wg32"][:2 * HID, 0:F2]
    wrp2 = arrs["wg32"][:2 * HID, F2:2 * F2]
    idx = arrs["idxu"].reshape(NCORES, 128, SD).astype(np.int64)
    cpk = arrs["cpack"][0]
    attsc1 = cpk[0:F1]
    b1r = cpk[F1:F1 + 128]
    attsc2 = cpk[F1 + 128:F1 + 128 + F2]
    b2r = cpk[F1 + 128 + F2:CP]
    batchv = arrs["wg32"][:, 2 * F2:2 * F2 + NT].reshape(NCORES, 128, NT)

    def edge_layer(tbl, xre, Fw, nheads, kpos, attsc, brow, h_w):
        h_all = np.zeros((NCORES, 128, NT * h_w), np.float32)
        for c in range(NCORES):
            for t in range(NT):
                d = D[t]
                A = tbl[idx[c, :, off[t]:off[t] + d].reshape(-1)].reshape(
                    128, d, Fw)
                xr = xre[c, :, t * Fw:(t + 1) * Fw]
                s = (A + xr[:, None, :]) * attsc[None, None, :]
                e = np.zeros((128, nheads, d), np.float32)
                for h in range(nheads):
                    base = h * HID
                    pos = np.abs(s[:, :, base:base + kpos[h]]).sum(2)
                    neg = np.abs(s[:, :, base + kpos[h]:base + HID]).sum(2)
                    attl = A[:, :, h_w + h] if Fw == F1 else A[:, :, HID + h]
                    attr = xr[:, (128 if Fw == F1 else HID) + h]
                    e[:, h] = (attl + attr[:, None]) + (pos - neg)
                p = np.exp(e)
                den = np.maximum(p.sum(2), 1e-30)
                outw = np.zeros((128, h_w), np.float32)
                for h in range(nheads):
                    outw[:, h * HID:(h + 1) * HID] = (
                        A[:, :, h * HID:(h + 1) * HID]
                        * p[:, h, :, None]).sum(1) / den[:, h:h + 1]
                hh = outw + brow[None, :h_w]
                hh = np.maximum(hh, np.exp(np.minimum(hh, 0.0)) - 1.0)
                h_all[c, :, t * h_w:(t + 1) * h_w] = hh
        return h_all

    tbl1 = np.zeros((TBL_N, F1), np.float32)
    xre1 = np.zeros((NCORES, 128, NT * F1), np.float32)
    for c in range(NCORES):
        for t in range(NT):
            xsl = xT[c][:, t * 128:(t + 1) * 128]
            tbl1[c * NSH + t * 128:c * NSH + (t + 1) * 128] = xsl.T @ wlp1
            xre1[c, :, t * F1:(t + 1) * F1] = xsl.T @ wrp1
    tbl1[np.arange(NCORES) * NSH + POISON, 128:130] = -1e4
    h1 = edge_layer(tbl1, xre1, F1, 2, k1, attsc1, cpk[F1:F1 + 128], 128)

    tbl2 = np.zeros((TBL_N, F2), np.float32)
    xre2 = np.zeros((NCORES, 128, NT * F2), np.float32)
    for c in range(NCORES):
        for t in range(NT):
            h1t = h1[c, :, t * 128:(t + 1) * 128]
            tbl2[c * NSH + t * 128:c * NSH + (t + 1) * 128] = h1t @ wlp2
            xre2[c, :, t * F2:(t + 1) * F2] = h1t @ wrp2
    tbl2[np.arange(NCORES) * NSH + POISON, HID:HID + 1] = -1e4
    h2 = edge_layer(tbl2, xre2, F2, 1, [k2], attsc2, b2r, HID)

    pooled = np.zeros((NCORES, NG, HID), np.float32)
    for c in range(NCORES):
        for t in range(NT):
            onehot = (np.arange(NG, dtype=np.float32)[None, :]
                      == batchv[c, :, t:t + 1]).astype(np.float32)
            pooled[c] += onehot.T @ h2[c, :, t * HID:(t + 1) * HID]
    return host_epilogue(pooled.sum(0), host_ctx)


# ---------------------------------------------------------------- device impl
def build_nc(static):
    import concourse.bass as bass
    import concourse.bacc as bacc
    import concourse.mybir as mybir
    import concourse.tile as tile
    from contextlib import ExitStack

    fp32 = mybir.dt.float32
    fp16 = mybir.dt.float16
    fp8 = mybir.dt.float8e4
    i32 = mybir.dt.int32
    u16 = mybir.dt.uint16
    AF = mybir.ActivationFunctionType
    OP = mybir.AluOpType

    D, SD = static["D"], static["SD"]
    off = np.concatenate([[0], np.cumsum(D)]).astype(np.int64)
    k1, k2 = static["k1"], static["k2"]

    nc = bacc.Bacc(None, num_devices=NCORES)

    # ---- I/O ----
    x8 = nc.dram_tensor("x8", [NSH, F_IN], fp8, kind="ExternalInput")
    wg16 = nc.dram_tensor("wg16", [F_IN, 2 * F1], fp16, kind="ExternalInput")
    wg32 = nc.dram_tensor("wg32", [128, 2 * F2 + NT], fp32,
                          kind="ExternalInput")
    idxu = nc.dram_tensor("idxu", [128, SD], u16, kind="ExternalInput")
    cpack = nc.dram_tensor("cpack", [1, CP], fp32, kind="ExternalInput")
    pooled_out = nc.dram_tensor("pooled", [NG, HID], fp32,
                                kind="ExternalOutput")

    # collective buffers (internal DRAM)
    tbl1_sh = nc.dram_tensor("tbl1_sh", [NSH, F1], fp32)
    tbl1 = nc.dram_tensor("tbl1", [TBL_N, F1], fp32, addr_space="Shared")
    tbl2_sh = nc.dram_tensor("tbl2_sh", [NSH, F2], fp32)
    tbl2 = nc.dram_tensor("tbl2", [TBL_N, F2], fp32, addr_space="Shared")
    pool_loc = nc.dram_tensor("pool_loc", [NG, HID], fp32)
    pool_red = nc.dram_tensor("pool_red", [NG, HID], fp32)

    with tile.TileContext(nc) as tc, ExitStack() as ctx:
        cp = ctx.enter_context(tc.tile_pool(name="const", bufs=1))
        wg16_s = cp.tile([F_IN, 2 * F1], fp16)
        nc.sync.dma_start(wg16_s[:], wg16[:, :])
        wlp1_s = wg16_s[:, 0:F1]
        wrp1_s = wg16_s[:, F1:2 * F1]
        wg32_s = cp.tile([128, 2 * F2 + NT], fp32)
        nc.sync.dma_start(wg32_s[:], wg32[:, :])
        wlp2_s = wg32_s[:, 0:F2]
        wrp2_s = wg32_s[:, F2:2 * F2]
        batch_s = wg32_s[:, 2 * F2:2 * F2 + NT]
        cpk_s = cp.tile([1, CP], fp32); nc.sync.dma_start(cpk_s[:], cpack[:, :])
        idxu_s = cp.tile([128, SD], u16); nc.sync.dma_start(idxu_s[:], idxu[:, :])
        idx32_s = cp.tile([128, SD], i32)
        nc.vector.tensor_scalar(idx32_s[:], idxu_s[:], 0, None, op0=OP.add)

        ones_s = cp.tile([1, 128], fp32); nc.vector.memset(ones_s[:], 1.0)
        pois_s = cp.tile([1, 2], fp32); nc.vector.memset(pois_s[:], -1e4)
        iotaF_i = cp.tile([128, 128], i32)
        nc.gpsimd.iota(iotaF_i[:], [[1, 128]], channel_multiplier=0)
        iotaP_i = cp.tile([128, 1], i32)
        nc.gpsimd.iota(iotaP_i[:], [[1, 1]], channel_multiplier=1)
        iotaF_f = cp.tile([128, 128], fp32)
        nc.vector.tensor_scalar(iotaF_f[:], iotaF_i[:], 0, None, op0=OP.add)
        iotaP_f = cp.tile([128, 1], fp32)
        nc.vector.tensor_scalar(iotaP_f[:], iotaP_i[:], 0, None, op0=OP.add)
        id_s = cp.tile([128, 128], fp32)
        nc.vector.tensor_scalar(id_s[:], iotaF_f[:], iotaP_f[:, 0:1], None,
                                op0=OP.is_equal)
        id16_s = cp.tile([128, 128], fp16)
        nc.vector.tensor_scalar(id16_s[:], id_s[:], 0.0, None, op0=OP.add)
        io64_s = iotaF_f[:, 0:NG]

        consts_s = cp.tile([128, CP], fp32)
        with tc.tile_pool(name="init_ps", bufs=1, space="PSUM") as ip:
            psC = ip.tile([128, CP], fp32)
            nc.tensor.matmul(psC[:], ones_s[:], cpk_s[:], start=True, stop=True)
            nc.scalar.copy(consts_s[:], psC[:])
        attsc1_s = consts_s[:, 0:F1]
        b1_s = consts_s[:, F1:F1 + 128]
        attsc2_s = consts_s[:, F1 + 128:F1 + 128 + F2]
        b2_s = consts_s[:, F1 + 128 + F2:CP]

        big = ctx.enter_context(tc.tile_pool(name="big", bufs=1))
        xre1_s = big.tile([128, NT * F1], fp32)
        h1_s = big.tile([128, NT * 128], fp32)

        # ---------------- phase A: layer-1 tables ----------------
        # x arrives [nodes, feat] fp8; widen to fp16 and PE-transpose per tile
        with tc.tile_pool(name="phA", bufs=3) as pa, \
             tc.tile_pool(name="phA_ps", bufs=3, space="PSUM") as pap:
            for t in range(NT):
                x8t = pa.tile([128, F_IN], fp8, tag="x8t")
                nc.sync.dma_start(x8t[:], x8[t * 128:(t + 1) * 128, :])
                x16t = pa.tile([128, F_IN], fp16, tag="x16t")
                nc.scalar.copy(x16t[:], x8t[:])
                psT = pap.tile([128, 128], fp16, tag="psT")
                nc.tensor.transpose(psT[:], x16t[:], id16_s[:])
                lhs = pa.tile([128, 128], fp16, tag="xTt")
                nc.scalar.copy(lhs[:], psT[:])
                ps = pap.tile([128, F1], fp32, tag="psA")
                nc.tensor.matmul(ps[:], lhs[:], wlp1_s, start=True, stop=True)
                stg = pa.tile([128, F1], fp32, tag="stgA")
                nc.scalar.copy(stg[:], ps[:])
                nc.sync.dma_start(tbl1_sh[t * 128:(t + 1) * 128, :], stg[:])
                ps2 = pap.tile([128, F1], fp32, tag="psA")
                nc.tensor.matmul(ps2[:], lhs[:], wrp1_s, start=True, stop=True)
                nc.scalar.copy(xre1_s[:, t * F1:(t + 1) * F1], ps2[:])
        nc.sync.dma_start(tbl1_sh[POISON:POISON + 1, 128:130], pois_s[0:1, 0:2])

        nc.gpsimd.collective_compute(
            "AllGather", mybir.AluOpType.bypass,
            replica_groups=[list(range(NCORES))],
            ins=[tbl1_sh[:, :]], outs=[tbl1[:, :]],
        )

        # ---------------- edge phase ----------------
        def edge_layer(tblT, xre_s, Fw, nheads, kpos, attsc_s, bt_s, h_out,
                       h_w):
            maxD = max(D)
            with tc.tile_pool(name=f"edg{Fw}", bufs=2) as pe, \
                 tc.tile_pool(name=f"sm{Fw}", bufs=3) as psm, \
                 tc.tile_pool(name=f"ps{Fw}", bufs=2, space="PSUM") as pps:
                for t in range(NT):
                    d = D[t]
                    # idx/out for the indirect DMA must be exact contiguous
                    # tiles (sliced/strided APs crash the DMA engine)
                    idxt = pe.tile([128, d], i32, tag="idxt")
                    nc.vector.tensor_scalar(
                        idxt[:], idx32_s[:, int(off[t]):int(off[t]) + d],
                        0, None, op0=OP.add)
                    A = pe.tile([128, d * Fw], fp32, tag="A")
                    # HW indirect DMA honors ONE offset per partition per call
                    for kk in range(d):
                        nc.gpsimd.indirect_dma_start(
                            out=A[:, kk * Fw:(kk + 1) * Fw],
                            out_offset=None,
                            in_=tblT[:, :],
                            in_offset=bass.IndirectOffsetOnAxis(
                                ap=idxt[:, kk:kk + 1], axis=0),
                        )
                    A3 = A[:].rearrange("p (d f) -> p d f", f=Fw)
                    xr = xre_s[:, t * Fw:(t + 1) * Fw]
                    xrb = xr.rearrange("p (o f) -> p o f", o=1).to_broadcast(
                        [128, d, Fw])
                    s = pe.tile([128, maxD * Fw], fp32, tag="s")
                    s3 = s[:, :d * Fw].rearrange("p (d f) -> p d f", f=Fw)
                    nc.vector.tensor_tensor(s3, A3, xrb, op=OP.add)
                    ascb = attsc_s.rearrange("p (o f) -> p o f",
                                             o=1).to_broadcast([128, d, Fw])
                    nc.vector.tensor_tensor(s3, s3, ascb, op=OP.mult)
                    # e-work tile: [pos_h, neg_h] x heads, then e [h, d]
                    ew = psm.tile([128, 4 * maxD], fp32, tag="ew")
                    for h in range(nheads):
                        base = h * HID
                        nc.vector.tensor_reduce(
                            ew[:, (2 * h) * d:(2 * h) * d + d],
                            s3[:, :, base:base + kpos[h]],
                            axis=mybir.AxisListType.X, op=OP.add,
                            apply_absolute_value=True)
                        nc.vector.tensor_reduce(
                            ew[:, (2 * h + 1) * d:(2 * h + 1) * d + d],
                            s3[:, :, base + kpos[h]:base + HID],
                            axis=mybir.AxisListType.X, op=OP.add,
                            apply_absolute_value=True)
                    # pn = pos - neg  -> [128, h, d]
                    pn = psm.tile([128, 2 * maxD], fp32, tag="pn")
                    ew4 = ew[:, :4 * d].rearrange("p (s d) -> p s d", d=d)
                    pnv = pn[:, :nheads * d].rearrange("p (s d) -> p s d", d=d)
                    nc.vector.tensor_tensor(
                        pnv, ew4[:, 0:2 * nheads:2, :],
                        ew4[:, 1:2 * nheads:2, :], op=OP.subtract)
                    # e = (attl + attr) + pn   (0.6 folded into watt cols)
                    ac = 128 if Fw == F1 else HID
                    attr = xr[:, ac:][:, :nheads]
                    attrb = bass.AP(attr.tensor, attr.offset,
                                    [attr.ap[0], [1, nheads], [0, d]])
                    attlv = bass.AP(A.tensor, A.offset + ac,
                                    [A.ap[0], [1, nheads], [Fw, d]])
                    tmp = psm.tile([128, 2 * maxD], fp32, tag="tmp")
                    tmpv = tmp[:, :nheads * d].rearrange("p (s d) -> p s d", d=d)
                    nc.vector.tensor_tensor(tmpv, attlv, attrb, op=OP.add)
                    ee = psm.tile([128, 2 * maxD], fp32, tag="ee")
                    eev = ee[:, :nheads * d].rearrange("p (s d) -> p s d", d=d)
                    nc.vector.tensor_tensor(eev, tmpv, pnv, op=OP.add)
                    # exp
                    pexp = psm.tile([128, 2 * maxD], fp32, tag="pexp")
                    pexpv = pexp[:, :nheads * d]
                    nc.scalar.activation(pexpv, ee[:, :nheads * d], AF.Exp)
                    pexp3 = pexpv.rearrange("p (s d) -> p s d", d=d)
                    # denom + clamp (all-pad rows sum to exactly 0) + recip
                    den = psm.tile([128, 2], fp32, tag="den")
                    nc.vector.tensor_reduce(den[:, :nheads], pexp3,
                                            axis=mybir.AxisListType.X,
                                            op=OP.add)
                    rd = psm.tile([128, 2], fp32, tag="rd")
                    nc.vector.tensor_scalar(rd[:, :nheads], den[:, :nheads],
                                            1e-30, None, op0=OP.max)
                    nc.vector.reciprocal(rd[:, :nheads], rd[:, :nheads])
                    # W = A * exp  (per head)
                    W = pe.tile([128, maxD * h_w], fp32, tag="W")
                    W3 = W[:, :d * h_w].rearrange("p (d f) -> p d f", f=h_w)
                    for h in range(nheads):
                        eb = bass.AP(pexp.tensor, pexp.offset + h * d,
                                     [pexp.ap[0], [1, d], [0, HID]])
                        nc.vector.tensor_tensor(
                            W3[:, :, h * HID:(h + 1) * HID],
                            A3[:, :, h * HID:(h + 1) * HID], eb, op=OP.mult)
                    # PSUM-accumulated identity matmuls over slots
                    po = pps.tile([128, h_w], fp32, tag="po")
                    for dd in range(d):
                        nc.tensor.matmul(po[:], id_s[:], W3[:, dd, :],
                                         start=(dd == 0), stop=(dd == d - 1))
                    # epilogue: divide by denom (ACT copy*scale), bias, elu
                    hh = psm.tile([128, h_w], fp32, tag="hh")
                    for h in range(nheads):
                        nc.scalar.activation(
                            hh[:, h * HID:(h + 1) * HID],
                            po[:, h * HID:(h + 1) * HID],
                            AF.Copy, bias=0.0, scale=rd[:, h:h + 1])
                    nc.vector.tensor_tensor(hh[:], hh[:], bt_s[:, :h_w],
                                            op=OP.add)
                    # elu: max(x, exp(min(x,0)) - 1)
                    mn = psm.tile([128, h_w], fp32, tag="mn")
                    nc.vector.tensor_scalar(mn[:], hh[:], 0.0, None, op0=OP.min)
                    ex = psm.tile([128, h_w], fp32, tag="ex")
                    nc.scalar.activation(ex[:], mn[:], AF.Exp)
                    nc.vector.scalar_tensor_tensor(
                        h_out[:, t * h_w:(t + 1) * h_w], ex[:], -1.0, hh[:],
                        op0=OP.add, op1=OP.max)

        edge_layer(tbl1, xre1_s, F1, 2, k1, attsc1_s, b1_s, h1_s, 128)

        # ---------------- phase C: layer-2 tables ----------------
        xre2_s = big.tile([128, NT * F2], fp32)
        with tc.tile_pool(name="phC", bufs=3) as pc, \
             tc.tile_pool(name="phC_ps", bufs=3, space="PSUM") as pcp:
            for t in range(NT):
                psT = pcp.tile([128, 128], fp32, tag="psT")
                nc.tensor.transpose(psT[:], h1_s[:, t * 128:(t + 1) * 128],
                                    id_s[:])
                h1T = pc.tile([128, 128], fp32, tag="h1T")
                nc.scalar.copy(h1T[:], psT[:])
                ps = pcp.tile([128, F2], fp32, tag="psC")
                nc.tensor.matmul(ps[:], h1T[:], wlp2_s, start=True, stop=True)
                stg = pc.tile([128, F2], fp32, tag="stgC")
                nc.scalar.copy(stg[:], ps[:])
                nc.sync.dma_start(tbl2_sh[t * 128:(t + 1) * 128, :], stg[:])
                ps2 = pcp.tile([128, F2], fp32, tag="psC")
                nc.tensor.matmul(ps2[:], h1T[:], wrp2_s, start=True, stop=True)
                nc.scalar.copy(xre2_s[:, t * F2:(t + 1) * F2], ps2[:])
        nc.sync.dma_start(tbl2_sh[POISON:POISON + 1, HID:HID + 1],
                          pois_s[0:1, 0:1])

        nc.gpsimd.collective_compute(
            "AllGather", mybir.AluOpType.bypass,
            replica_groups=[list(range(NCORES))],
            ins=[tbl2_sh[:, :]], outs=[tbl2[:, :]],
        )

        # ---------------- phase D: layer-2 edges ----------------
        h2_s = big.tile([128, NT * HID], fp32)
        edge_layer(tbl2, xre2_s, F2, 1, [k2], attsc2_s, b2_s, h2_s, HID)

        # ---------------- phase E: pooling + cross-core reduce ----------------
        with tc.tile_pool(name="phE", bufs=3) as pe_, \
             tc.tile_pool(name="phE_ps", bufs=1, space="PSUM") as pep:
            psP = pep.tile([NG, HID], fp32)
            for t in range(NT):
                oh = pe_.tile([128, NG], fp32, tag="oh")
                nc.vector.tensor_scalar(oh[:], io64_s, batch_s[:, t:t + 1],
                                        None, op0=OP.is_equal)
                nc.tensor.matmul(psP[:], oh[:], h2_s[:, t * HID:(t + 1) * HID],
                                 start=(t == 0), stop=(t == NT - 1))
            stg = pe_.tile([NG, HID], fp32, tag="stgE")
            nc.scalar.copy(stg[:], psP[:])
            nc.sync.dma_start(pool_loc[:, :], stg[:])
            # AllReduce -> every core holds the full pool; the host then
            # fetches a single shard (1 RTT instead of 8)
            nc.gpsimd.collective_compute(
                "AllReduce", mybir.AluOpType.add,
                replica_groups=[list(range(NCORES))],
                ins=[pool_loc[:, :]], outs=[pool_red[:, :]],
            )
            rstg = pe_.tile([NG, HID], fp32, tag="rstg")
            nc.sync.dma_start(rstg[:], pool_red[:, :])
            nc.sync.dma_start(pooled_out[:, :], rstg[:])

    nc.finalize()
    return nc


# ---------------------------------------------------------------- runner
class _Runner:
    """Builds the Bass module + shard_map'd jit executable ONCE; later calls
    reuse it (no retracing).  device_put is async -> callers overlap H2D with
    the rest of host prep."""

    def __init__(self, static):
        import jax
        import concourse.mybir as mybir
        from jax.sharding import Mesh, PartitionSpec, NamedSharding
        from jax.experimental.shard_map import shard_map
        from concourse.bass2jax import (
            _bass_exec_p, partition_id_tensor, install_neuronx_cc_hook)

        install_neuronx_cc_hook()
        self.jax = jax
        nc = build_nc(static)
        self.nc = nc
        pname = nc.partition_id_tensor.name if nc.partition_id_tensor else None
        in_names, out_names, out_avals, zero_shapes = [], [], [], []
        for alloc in nc.m.functions[0].allocations:
            if not isinstance(alloc, mybir.MemoryLocationSet):
                continue
            name = alloc.memorylocations[0].name
            if alloc.kind == "ExternalInput":
                if name != pname:
                    in_names.append(name)
            elif alloc.kind == "ExternalOutput":
                shape = tuple(alloc.tensor_shape)
                dtype = mybir.dt.np(alloc.dtype)
                out_names.append(name)
                out_avals.append(jax.core.ShapedArray(shape, dtype))
                zero_shapes.append((shape, dtype))
        self.dbg_name = None
        if nc.dbg_addr is not None:
            assert not nc.dbg_callbacks
            self.dbg_name = nc.dbg_addr.name
            in_names.append(self.dbg_name)
        n_params = len(in_names)
        all_names = in_names + out_names + ([pname] if pname else [])
        self.in_names = in_names
        self.out_names = out_names
        self.zero_shapes = zero_shapes
        donate = tuple(range(n_params, n_params + len(out_names)))

        def _body(*args):
            operands = list(args)
            if pname is not None:
                operands.append(partition_id_tensor())
            return tuple(_bass_exec_p.bind(
                *operands, out_avals=tuple(out_avals),
                in_names=tuple(all_names), out_names=tuple(out_names),
                lowering_input_output_aliases=(),
                sim_require_finite=True, sim_require_nnan=True, nc=nc))

        devices = jax.devices()[:NCORES]
        mesh = Mesh(np.asarray(devices), ("core",))
        self.sharding = NamedSharding(mesh, PartitionSpec("core"))
        nio = n_params + len(out_names)
        self.fn = jax.jit(
            shard_map(_body, mesh=mesh,
                      in_specs=(PartitionSpec("core"),) * nio,
                      out_specs=(PartitionSpec("core"),) * len(out_names),
                      check_rep=False),
            donate_argnums=donate, keep_unused=True)

    def put(self, arr):
        return self.jax.device_put(arr, self.sharding)

    def put_zeros(self):
        return [self.put(np.zeros((NCORES * s[0],) + tuple(s[1:]), dt))
                for s, dt in self.zero_shapes]

    def run(self, handles, zeros):
        if self.dbg_name is not None and self.dbg_name not in handles:
            handles[self.dbg_name] = self.put(
                np.zeros((NCORES, 2), np.uint32))
        outs = self.fn(*[handles[n] for n in self.in_names], *zeros)
        # pooled was AllReduced on device; shard 0 already holds the full sum
        pooled = outs[self.out_names.index("pooled")]
        return np.asarray(pooled.addressable_shards[0].data)


_CACHE = {}


def _get_runner(static):
    key = (tuple(static["D"]), tuple(static["k1"]), static["k2"])
    if key not in _CACHE:
        _CACHE[key] = _Runner(static)
    return _CACHE[key]


def kernel(**inputs) -> np.ndarray:
    w = _prep_weights(inputs)
    ei = np.asarray(inputs["edge_index"])
    src, dst, D, off, perm_nodes, rank_of = _prep_graph(ei)
    SD = int(D.sum())
    static = dict(D=[int(d) for d in D], SD=SD, k1=w["k1"], k2=w["k2"])
    runner = _get_runner(static)

    # big tensor first: device_put is async, the transfer streams while the
    # remaining host prep runs
    handles = {"x8": runner.put(_build_x8(inputs["x"], perm_nodes))}
    zeros = runner.put_zeros()
    handles["idxu"] = runner.put(_build_edges(src, dst, D, off, rank_of, SD))
    handles["wg16"] = runner.put(
        np.tile(np.concatenate([w["Wlp1"], w["Wrp1"]], 1)[None],
                (NCORES, 1, 1)).reshape(-1, 2 * F1))
    batch_np = np.asarray(inputs["batch"]).astype(np.int32)
    handles["wg32"] = runner.put(np.concatenate([
        np.tile(np.concatenate([w["Wlp2"], w["Wrp2"]], 1)[None],
                (NCORES, 1, 1)).reshape(-1, 2 * F2),
        _build_batch(batch_np, perm_nodes)], 1))
    handles["cpack"] = runner.put(np.tile(w["cpack"][None], (NCORES, 1)))

    pooled = runner.run(handles, zeros)
    host_ctx = dict(batch=batch_np, P2=w["P2"],
                    Wlin=np.asarray(inputs["Wlin"], np.float32),
                    blin=np.asarray(inputs["blin"], np.float32))
    return host_epilogue(pooled, host_ctx)
